# revision 26
# baseline (speedup 1.0000x reference)
"""GQA attention kernel for Trainium2 (8 NeuronCores).

Sharding: batch x head-group tensor parallel. Core c handles batch (c % 2)
and head group (c // 2): 8 q heads + 2 kv heads of that batch. Each core
computes its partial o-proj output (contraction over its 512 attn features);
the host sums the 4 partials per batch.

Device-side dataflow (per core):
  xT   [H=2048 hidden, S=2048 tokens] bf16  (x transposed on host)
  Q^T  [dim, tokens] per head-pair tile [128, S]   (projection with W
       stationary), RoPE'd via a fixed 128x128 rotation matmul + cos/sin.
  K^T  same, zero-padded into 4 [128, S] variants keyed (side, kv-head)
       so score matmuls can contract over the full 128 partitions.
  V    [tokens, dim] natural layout with an appended ones-column.
  scores S^T[kv, q] = K^T.T @ Q^T computed in 2-kv-tile groups into a
       bank-spanning PSUM tile [128, 1024]; exp'd with one ACT instr per
       group; only the diagonal 128x128 sub-block is masked (DVE mult by
       a fixed triangular 0/1 mask).
  PV   out[q, d+1] = pt.T @ V  (probs stationary, q tokens on PSUM
       partitions, streams only 65 columns) accumulated over kv tiles in
       a single PSUM accumulation group per (q-block, head). The ones
       column lands the softmax denominator at free position 64, so
       normalization is a per-partition reciprocal + tensor_scalar.
  attn^T for o-proj is rebuilt via XBAR DMA transposes (SBUF->SBUF) of
       the normalized [q, feature-pair] tiles.
  o-proj streams Wo with attn^T stationary; PSUM results staged to SBUF
       and DMA'd out per 128-token row. The 1/sqrt(64) scale is folded
       into Wq on the host.

The attention loop is ACT(exp)-throughput-bound, so ALL other PE work
(Q/K/V projections, o-proj) is chopped into sub-microsecond "pumps" and
woven between score groups to fill the PE stall slots while ACT drains
the exp backlog. Attention q-block qb only needs K/V token tiles up to
4*qb+3, so K proj of block tb feeds attention block tb as a pump one
block ahead.
"""

import os
import numpy as np
import ml_dtypes
from contextlib import ExitStack

import concourse.bass as bass
import concourse.tile as tile
from concourse import bacc
from concourse import mybir
from concourse import bass_utils

BF16 = mybir.dt.bfloat16
F32 = mybir.dt.float32
BF = ml_dtypes.bfloat16
AF = mybir.ActivationFunctionType
OP = mybir.AluOpType

H = 2048
S = 2048
B = 2
D = 64
QH = 8            # q heads per core
KVH = 2           # kv heads per core
QF = QH * D       # 512 q features per core
KF = KVH * D      # 128 kv features per core
NK = H // 128     # 16 contraction tiles
NT = S // 128     # 16 token tiles
QBS = 512         # q block size
NQB = S // QBS    # 4 q blocks
NPAIR = QF // 128 # 4 q head-pair tiles

_CACHE = {}


def tbc_(tb):
    return slice(tb * QBS, (tb + 1) * QBS)


def _build_program():
    nc = bacc.Bacc(
        "TRN2",
        target_bir_lowering=False,
        debug=False,
        enable_asserts=False,
        num_devices=8,
    )
    xT = nc.dram_tensor("xT", [H, S], BF16, kind="ExternalInput").ap()
    wqT = nc.dram_tensor("wqT", [H, QF], BF16, kind="ExternalInput").ap()
    wkT = nc.dram_tensor("wkT", [H, KF], BF16, kind="ExternalInput").ap()
    wvT = nc.dram_tensor("wvT", [H, KF], BF16, kind="ExternalInput").ap()
    woT = nc.dram_tensor("woT", [QF, H], BF16, kind="ExternalInput").ap()
    cost = nc.dram_tensor("cost", [128, S], BF16, kind="ExternalInput").ap()
    sint = nc.dram_tensor("sint", [128, S], BF16, kind="ExternalInput").ap()
    rotT = nc.dram_tensor("rotT", [128, 128], BF16, kind="ExternalInput").ap()
    trim = nc.dram_tensor("trim", [128, 128], BF16, kind="ExternalInput").ap()
    out = nc.dram_tensor("out", [S, H], F32, kind="ExternalOutput").ap()

    with tile.TileContext(nc) as tc:
        with ExitStack() as ctx:
            E = ctx.enter_context
            persist = E(tc.tile_pool(name="persist", bufs=1))
            psS = E(tc.tile_pool(name="psS", bufs=2, space="PSUM"))
            psQ = E(tc.tile_pool(name="psQ", bufs=1, space="PSUM"))
            psV = E(tc.tile_pool(name="psV", bufs=2, space="PSUM"))
            psO = E(tc.tile_pool(name="psO", bufs=1, space="PSUM"))
            wk = E(tc.tile_pool(name="wk", bufs=3))
            wk2 = E(tc.tile_pool(name="wk2", bufs=2))
            aq = E(tc.tile_pool(name="aq", bufs=8))
            stg = E(tc.tile_pool(name="stg", bufs=2))

            # ---------------- batched loads ----------------
            # DMA device is the startup bottleneck; order/chunk loads so
            # K proj / V proj / first Q projs start as early as possible.
            wk_all = persist.tile([128, NK, KF], BF16, tag="wk_all")
            wkr = wkT.rearrange("(k p) c -> p k c", p=128)
            nc.sync.dma_start(wk_all[:, 0:8, :], wkr[:, 0:8, :])
            xa = persist.tile([128, NK, S], BF16, tag="xa")
            xr = xT.rearrange("(k p) s -> p k s", p=128)
            nc.sync.dma_start(xa[:, 0:4, tbc_(0)], xr[:, 0:4, tbc_(0)])
            nc.sync.dma_start(wk_all[:, 8:16, :], wkr[:, 8:16, :])
            nc.sync.dma_start(xa[:, 4:8, tbc_(0)], xr[:, 4:8, tbc_(0)])
            wv_all = persist.tile([128, NK, KF], BF16, tag="wv_all")
            nc.sync.dma_start(
                wv_all[:], wvT.rearrange("(k p) c -> p k c", p=128))
            for kc in range(8, NK, 4):
                nc.sync.dma_start(
                    xa[:, kc:kc + 4, tbc_(0)], xr[:, kc:kc + 4, tbc_(0)])
            rt = persist.tile([128, 128], BF16, tag="rt")
            nc.sync.dma_start(rt[:], rotT[:, :])
            cs = persist.tile([128, S], BF16, tag="cs")
            nc.sync.dma_start(cs[:], cost[:, :])
            sn = persist.tile([128, S], BF16, tag="sn")
            nc.sync.dma_start(sn[:], sint[:, :])
            tri = persist.tile([128, 128], BF16, tag="tri")
            nc.sync.dma_start(tri[:], trim[:, :])
            wq_all = persist.tile([128, NK, QF], BF16, tag="wq_all")
            wqr = wqT.rearrange("(k p) c -> p k c", p=128)
            nc.sync.dma_start(wq_all[:, :, 0:256], wqr[:, :, 0:256])
            nc.sync.dma_start(wq_all[:, :, 256:512], wqr[:, :, 256:512])
            for tb in range(1, NQB):
                nc.sync.dma_start(xa[:, :, tbc_(tb)], xr[:, :, tbc_(tb)])
            wo_sb = []
            for p in range(NPAIR):
                t = persist.tile([128, H], BF16, tag=f"wo{p}", name=f"wo{p}")
                nc.sync.dma_start(t[:], woT[p * 128:(p + 1) * 128, :])
                wo_sb.append(t)

            # ---------------- persistent activation tiles ----------------
            qt_sb = [persist.tile([128, S], BF16, tag=f"qt{p}", name=f"qt{p}") for p in range(NPAIR)]
            ktp = {(sd, v): persist.tile([128, S], BF16, tag=f"ktp{sd}{v}", name=f"ktp{sd}{v}")
                   for sd in (0, 1) for v in (0, 1)}
            va = [persist.tile([128, NT, D + 1], BF16, tag=f"va{v}", name=f"va{v}") for v in (0, 1)]
            att = [persist.tile([128, S], BF16, tag=f"att{p}", name=f"att{p}") for p in range(NPAIR)]

            # zero pads (Pool engine; it is otherwise idle, and these have
            # no input dependencies so they run during the initial loads)
            nc.gpsimd.memset(ktp[(0, 0)][64:128, :], 0.0)
            nc.gpsimd.memset(ktp[(1, 1)][0:64, :], 0.0)
            nc.gpsimd.memset(ktp[(1, 0)][0:64, :], 0.0)
            nc.gpsimd.memset(ktp[(0, 1)][64:128, :], 0.0)
            nc.gpsimd.memset(va[0][:, :, D:D + 1], 1.0)
            nc.gpsimd.memset(va[1][:, :, D:D + 1], 1.0)

            def rope(ps, tb, outs):
                """ps: psum [128,512] raw pre-RoPE projection (feature-major).
                outs: list of (row_slice, out_ap) receiving rotated bf16."""
                raw = wk.tile([128, QBS], BF16, tag="rope_raw")
                nc.gpsimd.tensor_copy(out=raw[:], in_=ps[:])
                rp = psQ.tile([128, QBS], F32, tag="q")
                nc.tensor.matmul(rp[:], lhsT=rt[:], rhs=raw[:], start=True, stop=True)
                t1 = wk.tile([128, QBS], BF16, tag="rope_t1")
                nc.vector.tensor_tensor(out=t1[:], in0=rp[:], in1=sn[:, tbc_(tb)], op=OP.mult)
                t2 = wk.tile([128, QBS], BF16, tag="rope_t2")
                nc.vector.tensor_tensor(out=t2[:], in0=raw[:], in1=cs[:, tbc_(tb)], op=OP.mult)
                for rows, out_ap in outs:
                    nc.vector.tensor_tensor(
                        out=out_ap, in0=t1[rows, :], in1=t2[rows, :], op=OP.add)

            def kfinish(kp, tb):
                """RoPE K proj block tb into the padded variants + mirror."""
                rope(kp, tb, [
                    (slice(0, 64), ktp[(0, 0)][0:64, tbc_(tb)]),
                    (slice(64, 128), ktp[(1, 1)][64:128, tbc_(tb)]),
                ])
                nc.sync.dma_start(ktp[(1, 0)][64:128, tbc_(tb)],
                                  ktp[(0, 0)][0:64, tbc_(tb)])
                nc.sync.dma_start(ktp[(0, 1)][0:64, tbc_(tb)],
                                  ktp[(1, 1)][64:128, tbc_(tb)])

            def vproj(t):
                vp = psQ.tile([128, KF], F32, tag="q")
                for k in range(NK):
                    nc.tensor.matmul(
                        vp[:], lhsT=xa[:, k, t * 128:(t + 1) * 128],
                        rhs=wv_all[:, k, :],
                        start=(k == 0), stop=(k == NK - 1))
                for v in (0, 1):
                    nc.gpsimd.tensor_copy(
                        out=va[v][:, t, 0:D], in_=vp[:, v * D:(v + 1) * D])

            # ---------------- pump machinery ----------------
            # Each pump is a closure emitting ~0.2-0.9us of PE work; pumps
            # are interleaved between attention score groups. All
            # projection PSUM flows through the single psQ bank: within a
            # generator the accumulator's reader (rope raw copy / va copy)
            # is emitted before the next psQ allocation, so the pool
            # rotation never stalls.
            def qproj_pumps(p, tb):
                st = {}
                def mk(kk):
                    def pump():
                        if kk == 0:
                            st["qp"] = psQ.tile([128, QBS], F32, tag="q",
                                                name=f"qp{p}_{tb}")
                        for k in (kk, kk + 1):
                            nc.tensor.matmul(
                                st["qp"][:],
                                lhsT=wq_all[:, k, p * 128:(p + 1) * 128],
                                rhs=xa[:, k, tbc_(tb)],
                                start=(k == 0), stop=(k == NK - 1))
                    return pump
                pumps = [mk(kk) for kk in range(0, NK, 2)]
                pumps.append(lambda: rope(
                    st["qp"], tb, [(slice(0, 128), qt_sb[p][:, tbc_(tb)])]))
                return pumps

            def kproj_pumps(tb):
                st = {}
                def mk(kk):
                    def pump():
                        if kk == 0:
                            st["kp"] = psQ.tile([128, QBS], F32, tag="q",
                                                name=f"kp{tb}")
                        for k in range(kk, kk + 4):
                            nc.tensor.matmul(
                                st["kp"][:], lhsT=wk_all[:, k, :],
                                rhs=xa[:, k, tbc_(tb)],
                                start=(k == 0), stop=(k == NK - 1))
                    return pump
                pumps = [mk(kk) for kk in range(0, NK, 4)]
                pumps.append(lambda: kfinish(st["kp"], tb))
                return pumps

            def oproj_pumps(t, last_split=False, pool=None):
                st = {}
                opool = pool if pool is not None else psO
                def mk(n):
                    def pump():
                        if n == 0:
                            st["so"] = stg.tile([128, H], F32, tag="stg",
                                                name=f"so{t}")
                        op_ps = opool.tile([128, 512], F32,
                                           tag="op" if opool is psO else "ps")
                        for p2 in range(NPAIR):
                            nc.tensor.matmul(
                                op_ps[:], lhsT=att[p2][:, t * 128:(t + 1) * 128],
                                rhs=wo_sb[p2][:, n * 512:(n + 1) * 512],
                                start=(p2 == 0), stop=(p2 == NPAIR - 1))
                        if n % 2 == 0:
                            nc.vector.tensor_copy(
                                out=st["so"][:, n * 512:(n + 1) * 512], in_=op_ps[:])
                        else:
                            nc.scalar.activation(
                                st["so"][:, n * 512:(n + 1) * 512], op_ps[:], AF.Copy)
                        if last_split and n in (1, 3):
                            nc.sync.dma_start(
                                out[t * 128:(t + 1) * 128,
                                    (n - 1) * 512:(n + 1) * 512],
                                st["so"][:, (n - 1) * 512:(n + 1) * 512])
                        elif n == 3:
                            nc.sync.dma_start(
                                out[t * 128:(t + 1) * 128, :], st["so"][:])
                    return pump
                return [mk(n) for n in range(4)]

            def interleave_nonadjacent(big, small):
                """Alternate big (chunky, shared-psum) pumps with small ones
                so consecutive big pumps never contend for their single
                PSUM buffer; pad with no-op spacers once small runs out so
                two bigs never land in the same drain slot."""
                late = 10 ** 9
                res = []
                bi, si = 0, 0
                while bi < len(big) or si < len(small):
                    if si < len(small):
                        res.append(small[si]); si += 1
                    elif bi < len(big) and res and res[-1][0] is not None:
                        res.append((None, late))
                    if bi < len(big):
                        res.append(big[bi]); bi += 1
                return res

            def build_pumps(qb):
                """Returns [(pump, deadline_sg)] for attention block qb.
                Deadlines pin same-block Q pairs before the heads that read
                them; everything else only needs to land by block end."""
                ngrp = 2 * qb + 2
                late = 10 ** 9
                qp = []
                # drains happen at the bottom of each score-group slot, so
                # a deadline of N-1 completes before slot N's score matmul
                qp += [(f, 4 * ngrp - 1) for f in qproj_pumps(2, qb)]
                if qb + 1 < NQB:
                    qp += [(f, late) for f in kproj_pumps(qb + 1)]
                qp += [(f, 6 * ngrp - 1) for f in qproj_pumps(3, qb)]
                if qb < 3:
                    qp += [(lambda t=t: vproj(t), late)
                           for t in range(4 * qb + 4, 4 * qb + 8)]
                if qb + 1 < NQB:
                    qp += [(f, late) for f in qproj_pumps(0, qb + 1)]
                    qp += [(f, late) for f in qproj_pumps(1, qb + 1)]
                og = []
                if qb == 2:
                    for t in range(0, 4):
                        og += [(f, late) for f in oproj_pumps(t)]
                elif qb == 3:
                    for t in range(4, 12):
                        og += [(f, late) for f in oproj_pumps(t)]
                if og:
                    return interleave_nonadjacent(og, qp)
                return qp

            # ---------------- phase 0 ----------------
            # K proj block 0 through psS (scores have not started), then
            # V tiles 0..3 and the first two Q pairs through psQ.
            kp0 = psS.tile([128, QBS], F32, tag="ps", name="kp0")
            for k in range(NK):
                nc.tensor.matmul(
                    kp0[:], lhsT=wk_all[:, k, :], rhs=xa[:, k, tbc_(0)],
                    start=(k == 0), stop=(k == NK - 1))
            for t in range(4):
                vproj(t)
            kfinish(kp0, 0)
            for pump in qproj_pumps(0, 0) + qproj_pumps(1, 0):
                pump()

            # ---------------- attention ----------------
            for qb in range(NQB):
                pumps = build_pumps(qb)
                # suffix-min of deadlines so a due pump forces everything
                # queued in front of it out as well
                due = [0] * (len(pumps) + 1)
                due[len(pumps)] = 10 ** 9
                for i in range(len(pumps) - 1, -1, -1):
                    due[i] = min(pumps[i][1], due[i + 1])
                n_sg = QH * (2 * qb + 2)
                pi = 0
                attq_cur = None
                for hh in range(QH):
                    p = hh // 2
                    half = hh % 2
                    v = hh // 4
                    ksel = ktp[(half, v)]
                    nkv = 4 * qb + 4
                    ngrp = nkv // 2
                    if half == 0:
                        attq_cur = [aq.tile([128, 128], BF16, tag="attq",
                                            name=f"attq{qb}_{p}_{s}")
                                    for s in range(4)]
                    ov = psV.tile([128, 4, D + 1], F32, tag="ov")
                    pts = []
                    for g2 in range(ngrp):
                        sc = psS.tile([128, 2 * QBS], F32, tag="ps")
                        pt = wk.tile([128, 2 * QBS], BF16, tag="pt")
                        j0 = 2 * g2 - 4 * qb
                        for ht in (0, 1):
                            kv = 2 * g2 + ht
                            j = kv - 4 * qb
                            c0 = 128 * j if j > 0 else 0
                            base = ht * QBS
                            nc.tensor.matmul(
                                sc[:, base + c0:base + QBS],
                                lhsT=ksel[:, kv * 128:(kv + 1) * 128],
                                rhs=qt_sb[p][:, qb * QBS + c0:(qb + 1) * QBS],
                                start=True, stop=True)
                        if j0 + 1 < 0:
                            nc.scalar.activation(pt[:], sc[:], AF.Exp)
                        else:
                            for ht in (0, 1):
                                j = 2 * g2 + ht - 4 * qb
                                c0 = 128 * j if j > 0 else 0
                                base = ht * QBS
                                nc.scalar.activation(
                                    pt[:, base + c0:base + QBS],
                                    sc[:, base + c0:base + QBS], AF.Exp)
                        for ht in (0, 1):
                            j = 2 * g2 + ht - 4 * qb
                            if j >= 0:
                                c0 = 128 * j
                                base = ht * QBS
                                nc.vector.tensor_tensor(
                                    out=pt[:, base + c0:base + c0 + 128],
                                    in0=pt[:, base + c0:base + c0 + 128],
                                    in1=tri[:], op=OP.mult)
                        pts.append(pt)
                        if g2 > 0:
                            _pv_group(nc, ov, pts[g2 - 1], va[v], qb, g2 - 1, nkv)
                        # self-correcting pacing: spread the remaining pumps
                        # evenly over the remaining score-group slots, and
                        # force any whose deadline is due
                        sg_idx = hh * ngrp + g2
                        sgs_left = n_sg - sg_idx
                        want = -((pi - len(pumps)) // sgs_left)  # ceil div
                        emitted = 0
                        while pi < len(pumps) and (
                                emitted < want or due[pi] <= sg_idx):
                            if pumps[pi][0] is not None:
                                pumps[pi][0]()
                            pi += 1
                            emitted += 1
                    _pv_group(nc, ov, pts[ngrp - 1], va[v], qb, ngrp - 1, nkv)
                    # normalize: denominator sits at free position 64
                    rec = wk2.tile([128, 4], F32, tag="rec")
                    nc.vector.reciprocal(rec[:, :], ov[:, :, D])
                    for s in range(4):
                        nc.vector.tensor_scalar(
                            out=attq_cur[s][:, half * D:(half + 1) * D],
                            in0=ov[:, s, 0:D],
                            scalar1=rec[:, s:s + 1], scalar2=None, op0=OP.mult)
                    if half == 1:
                        for s in range(4):
                            nc.sync.dma_start_transpose(
                                att[p][:, (qb * 4 + s) * 128:(qb * 4 + s + 1) * 128],
                                attq_cur[s][:])
                while pi < len(pumps):
                    if pumps[pi][0] is not None:
                        pumps[pi][0]()
                    pi += 1
            # trailing o-proj: scores are done, so the psS banks are free
            # and give the accumulators double-buffering
            for t in range(12, 16):
                for pump in oproj_pumps(t, last_split=(t == 15), pool=psS):
                    pump()
    nc.compile()
    return nc


def _pv_group(nc, ov, pt, vat, qb, g2, nkv):
    """Accumulate PV matmuls for score group g2 (kv tiles 2*g2, 2*g2+1)."""
    for ht in (0, 1):
        kv = 2 * g2 + ht
        j = kv - 4 * qb
        for s in range(4):
            if j > s:
                continue  # this kv tile is fully masked for q subtile s
            nc.tensor.matmul(
                ov[:, s, :],
                lhsT=pt[:, ht * QBS + s * 128:ht * QBS + (s + 1) * 128],
                rhs=vat[:, kv, :],
                start=(kv == 0 and s == 0),
                stop=(kv == nkv - 1 and s == 3))


def _host_tables():
    freq = 1.0 / (10000.0 ** (np.arange(0, D, 2, dtype=np.float64) / D))
    t = np.arange(S, dtype=np.float64)
    fr = t[:, None] * freq[None, :]                       # (S, 32)
    emb = np.concatenate([fr, fr], axis=-1)               # (S, 64)
    cos64 = np.cos(emb).T.astype(np.float32)              # (64, S)
    sin64 = np.sin(emb).T.astype(np.float32)
    cos128 = np.concatenate([cos64, cos64], axis=0).astype(BF)
    sin128 = np.concatenate([sin64, sin64], axis=0).astype(BF)
    R = np.zeros((64, 64), np.float32)
    R[np.arange(32), 32 + np.arange(32)] = -1.0
    R[32 + np.arange(32), np.arange(32)] = 1.0
    R128 = np.zeros((128, 128), np.float32)
    R128[:64, :64] = R
    R128[64:, 64:] = R
    rotT = np.ascontiguousarray(R128.T).astype(BF)
    r = np.arange(128)[:, None]
    c = np.arange(128)[None, :]
    trimask = (r <= c).astype(np.float32).astype(BF)
    return cos128, sin128, rotT, trimask


def _make_in_maps(inputs):
    x = np.asarray(inputs["x"], np.float32)
    Wq = np.asarray(inputs["Wq"], np.float32)
    Wk = np.asarray(inputs["Wk"], np.float32)
    Wv = np.asarray(inputs["Wv"], np.float32)
    Wo = np.asarray(inputs["Wo"], np.float32)
    cos128, sin128, rotT, trimask = _host_tables()
    in_maps = []
    for core in range(8):
        g, b = core // 2, core % 2
        im = {
            "xT": np.ascontiguousarray(x[b].T).astype(BF),
            "wqT": np.ascontiguousarray((Wq[QF * g:QF * (g + 1), :] / 8.0).T).astype(BF),
            "wkT": np.ascontiguousarray(Wk[KF * g:KF * (g + 1), :].T).astype(BF),
            "wvT": np.ascontiguousarray(Wv[KF * g:KF * (g + 1), :].T).astype(BF),
            "woT": np.ascontiguousarray(Wo[:, QF * g:QF * (g + 1)].T).astype(BF),
            "cost": cos128,
            "sint": sin128,
            "rotT": rotT,
            "trim": trimask,
        }
        in_maps.append(im)
    return in_maps


def kernel(x, Wq, Wk, Wv, Wo):
    if "nc" not in _CACHE:
        _CACHE["nc"] = _build_program()
    nc = _CACHE["nc"]

    in_maps = _make_in_maps(
        {"x": x, "Wq": Wq, "Wk": Wk, "Wv": Wv, "Wo": Wo})

    trace = bool(int(os.environ.get("KERNEL_TRACE", "0")))
    res = bass_utils.run_bass_kernel_spmd(
        nc, in_maps, core_ids=list(range(8)), trace=trace)
    _CACHE["last_result"] = res

    out = np.zeros((B, S, H), np.float32)
    for core in range(8):
        g, b = core // 2, core % 2
        out[b] += np.asarray(res.results[core]["out"], np.float32)
    return out


# revision 39
# speedup vs baseline: 1.0429x; 1.0429x over previous
"""GQA attention kernel for Trainium2 (8 NeuronCores).

Sharding: batch x head-group tensor parallel. Core c handles batch (c % 2)
and head group (c // 2): 8 q heads + 2 kv heads of that batch. Each core
computes its partial o-proj output (contraction over its 512 attn features);
the host sums the 4 partials per batch.

Device-side dataflow (per core):
  xT   [H=2048 hidden, S=2048 tokens] bf16  (x transposed on host)
  Q^T  [dim, tokens] per head-pair tile [128, S]   (projection with W
       stationary), RoPE'd via a fixed 128x128 rotation matmul + cos/sin.
  K^T  same, zero-padded into 4 [128, S] variants keyed (side, kv-head)
       so score matmuls can contract over the full 128 partitions.
  V    [tokens, dim] natural layout with an appended ones-column.
  scores S^T[kv, q] = K^T.T @ Q^T computed in 2-kv-tile groups into a
       bank-spanning PSUM tile [128, 1024]; exp'd with one ACT instr per
       group; only the diagonal 128x128 sub-block is masked (DVE mult by
       a fixed triangular 0/1 mask).
  PV   out[q, d+1] = pt.T @ V  (probs stationary, q tokens on PSUM
       partitions, streams only 65 columns) accumulated over kv tiles in
       a single PSUM accumulation group per (q-block, head). The ones
       column lands the softmax denominator at free position 64, so
       normalization is a per-partition reciprocal + tensor_scalar.
  attn^T for o-proj is rebuilt via XBAR DMA transposes (SBUF->SBUF) of
       the normalized [q, feature-pair] tiles.
  o-proj streams Wo with attn^T stationary; PSUM results staged to SBUF
       and DMA'd out per 128-token row. The 1/sqrt(64) scale is folded
       into Wq on the host.

The attention loop is ACT(exp)-throughput-bound, so ALL other PE work
(Q/K/V projections, o-proj) is chopped into sub-microsecond "pumps" and
woven between score groups to fill the PE stall slots while ACT drains
the exp backlog. Attention q-block qb only needs K/V token tiles up to
4*qb+3, so K proj of block tb feeds attention block tb as a pump one
block ahead.
"""

import os
import numpy as np
import ml_dtypes
from contextlib import ExitStack

import concourse.bass as bass
import concourse.tile as tile
from concourse import bacc
from concourse import mybir
from concourse import bass_utils

BF16 = mybir.dt.bfloat16
F32 = mybir.dt.float32
BF = ml_dtypes.bfloat16
AF = mybir.ActivationFunctionType
OP = mybir.AluOpType

H = 2048
S = 2048
B = 2
D = 64
QH = 8            # q heads per core
KVH = 2           # kv heads per core
QF = QH * D       # 512 q features per core
KF = KVH * D      # 128 kv features per core
NK = H // 128     # 16 contraction tiles
NT = S // 128     # 16 token tiles
QBS = 512         # q block size
NQB = S // QBS    # 4 q blocks
NPAIR = QF // 128 # 4 q head-pair tiles

_CACHE = {}


def tbc_(tb):
    return slice(tb * QBS, (tb + 1) * QBS)


def _build_program():
    nc = bacc.Bacc(
        "TRN2",
        target_bir_lowering=False,
        debug=False,
        enable_asserts=False,
        num_devices=8,
    )
    xT = nc.dram_tensor("xT", [H, S], BF16, kind="ExternalInput").ap()
    wqT = nc.dram_tensor("wqT", [H, QF], BF16, kind="ExternalInput").ap()
    wkT = nc.dram_tensor("wkT", [H, KF], BF16, kind="ExternalInput").ap()
    wvT = nc.dram_tensor("wvT", [H, KF], BF16, kind="ExternalInput").ap()
    woT = nc.dram_tensor("woT", [QF, H], BF16, kind="ExternalInput").ap()
    cost = nc.dram_tensor("cost", [128, S], BF16, kind="ExternalInput").ap()
    sint = nc.dram_tensor("sint", [128, S], BF16, kind="ExternalInput").ap()
    rotT = nc.dram_tensor("rotT", [128, 128], BF16, kind="ExternalInput").ap()
    trim = nc.dram_tensor("trim", [128, 128], BF16, kind="ExternalInput").ap()
    out = nc.dram_tensor("out", [S, H], BF16, kind="ExternalOutput").ap()

    with tile.TileContext(nc) as tc:
        with ExitStack() as ctx:
            E = ctx.enter_context
            persist = E(tc.tile_pool(name="persist", bufs=1))
            psS = E(tc.tile_pool(name="psS", bufs=2, space="PSUM"))
            psQ = E(tc.tile_pool(name="psQ", bufs=1, space="PSUM"))
            psV = E(tc.tile_pool(name="psV", bufs=2, space="PSUM"))
            psO = E(tc.tile_pool(name="psO", bufs=1, space="PSUM"))
            wk = E(tc.tile_pool(name="wk", bufs=3))
            wk2 = E(tc.tile_pool(name="wk2", bufs=2))
            aq = E(tc.tile_pool(name="aq", bufs=8))
            stg = E(tc.tile_pool(name="stg", bufs=2))

            # ---------------- batched loads ----------------
            # DMA device is the startup bottleneck; order/chunk loads so
            # K proj / V proj / first Q projs start as early as possible.
            wk_all = persist.tile([128, NK, KF], BF16, tag="wk_all")
            wkr = wkT.rearrange("(k p) c -> p k c", p=128)
            nc.sync.dma_start(wk_all[:, 0:8, :], wkr[:, 0:8, :])
            xa = persist.tile([128, NK, S], BF16, tag="xa")
            xr = xT.rearrange("(k p) s -> p k s", p=128)
            nc.sync.dma_start(xa[:, 0:4, tbc_(0)], xr[:, 0:4, tbc_(0)])
            nc.sync.dma_start(wk_all[:, 8:16, :], wkr[:, 8:16, :])
            nc.sync.dma_start(xa[:, 4:8, tbc_(0)], xr[:, 4:8, tbc_(0)])
            wv_all = persist.tile([128, NK, KF], BF16, tag="wv_all")
            nc.sync.dma_start(
                wv_all[:], wvT.rearrange("(k p) c -> p k c", p=128))
            for kc in range(8, NK, 4):
                nc.sync.dma_start(
                    xa[:, kc:kc + 4, tbc_(0)], xr[:, kc:kc + 4, tbc_(0)])
            rt = persist.tile([128, 128], BF16, tag="rt")
            nc.sync.dma_start(rt[:], rotT[:, :])
            cs = persist.tile([128, S], BF16, tag="cs")
            nc.sync.dma_start(cs[:], cost[:, :])
            sn = persist.tile([128, S], BF16, tag="sn")
            nc.sync.dma_start(sn[:], sint[:, :])
            tri = persist.tile([128, 128], BF16, tag="tri")
            nc.sync.dma_start(tri[:], trim[:, :])
            wq_all = persist.tile([128, NK, QF], BF16, tag="wq_all")
            wqr = wqT.rearrange("(k p) c -> p k c", p=128)
            nc.sync.dma_start(wq_all[:, :, 0:256], wqr[:, :, 0:256])
            nc.sync.dma_start(wq_all[:, :, 256:512], wqr[:, :, 256:512])
            for tb in range(1, NQB):
                nc.sync.dma_start(xa[:, :, tbc_(tb)], xr[:, :, tbc_(tb)])
            wo_sb = []
            for p in range(NPAIR):
                t = persist.tile([128, H], BF16, tag=f"wo{p}", name=f"wo{p}")
                nc.sync.dma_start(t[:], woT[p * 128:(p + 1) * 128, :])
                wo_sb.append(t)

            # ---------------- persistent activation tiles ----------------
            qt_sb = [persist.tile([128, S], BF16, tag=f"qt{p}", name=f"qt{p}") for p in range(NPAIR)]
            ktp = {(sd, v): persist.tile([128, S], BF16, tag=f"ktp{sd}{v}", name=f"ktp{sd}{v}")
                   for sd in (0, 1) for v in (0, 1)}
            va = [persist.tile([128, NT, D + 1], BF16, tag=f"va{v}", name=f"va{v}") for v in (0, 1)]
            att = [persist.tile([128, S], BF16, tag=f"att{p}", name=f"att{p}") for p in range(NPAIR)]

            # zero pads (Pool engine; it is otherwise idle, and these have
            # no input dependencies so they run during the initial loads)
            nc.gpsimd.memset(ktp[(0, 0)][64:128, :], 0.0)
            nc.gpsimd.memset(ktp[(1, 1)][0:64, :], 0.0)
            nc.gpsimd.memset(ktp[(1, 0)][0:64, :], 0.0)
            nc.gpsimd.memset(ktp[(0, 1)][64:128, :], 0.0)
            nc.gpsimd.memset(va[0][:, :, D:D + 1], 1.0)
            nc.gpsimd.memset(va[1][:, :, D:D + 1], 1.0)

            def rope_pre(ps):
                """Drain the projection accumulator to SBUF (releases the
                psQ bank); the rest of RoPE runs in rope_post."""
                raw = wk.tile([128, QBS], BF16, tag="rope_raw")
                nc.vector.tensor_copy(out=raw[:], in_=ps[:])
                return raw

            def rope_post(raw, tb, outs):
                rp = psQ.tile([128, QBS], F32, tag="q")
                nc.tensor.matmul(rp[:], lhsT=rt[:], rhs=raw[:], start=True, stop=True)
                t1 = wk.tile([128, QBS], BF16, tag="rope_t1")
                nc.vector.tensor_tensor(out=t1[:], in0=rp[:], in1=sn[:, tbc_(tb)], op=OP.mult)
                t2 = wk.tile([128, QBS], BF16, tag="rope_t2")
                nc.vector.tensor_tensor(out=t2[:], in0=raw[:], in1=cs[:, tbc_(tb)], op=OP.mult)
                for rows, out_ap in outs:
                    nc.vector.tensor_tensor(
                        out=out_ap, in0=t1[rows, :], in1=t2[rows, :], op=OP.add)

            def rope(ps, tb, outs):
                rope_post(rope_pre(ps), tb, outs)

            def kfinish(kp, tb):
                """RoPE K proj block tb into the padded variants + mirror."""
                rope(kp, tb, [
                    (slice(0, 64), ktp[(0, 0)][0:64, tbc_(tb)]),
                    (slice(64, 128), ktp[(1, 1)][64:128, tbc_(tb)]),
                ])
                nc.sync.dma_start(ktp[(1, 0)][64:128, tbc_(tb)],
                                  ktp[(0, 0)][0:64, tbc_(tb)])
                nc.sync.dma_start(ktp[(0, 1)][0:64, tbc_(tb)],
                                  ktp[(1, 1)][64:128, tbc_(tb)])

            def vproj(t, pool=None, tag="q"):
                vp = (pool or psQ).tile([128, KF], F32, tag=tag)
                for k in range(NK):
                    nc.tensor.matmul(
                        vp[:], lhsT=xa[:, k, t * 128:(t + 1) * 128],
                        rhs=wv_all[:, k, :],
                        start=(k == 0), stop=(k == NK - 1))
                for v in (0, 1):
                    nc.vector.tensor_copy(
                        out=va[v][:, t, 0:D], in_=vp[:, v * D:(v + 1) * D])

            # ---------------- pump machinery ----------------
            # Each pump is a closure emitting ~0.2-0.9us of PE work; pumps
            # are interleaved between attention score groups. All
            # projection PSUM flows through the single psQ bank: within a
            # generator the accumulator's reader (rope raw copy / va copy)
            # is emitted before the next psQ allocation, so the pool
            # rotation never stalls.
            def qproj_pumps(p, tb):
                st = {}
                def mk(kk):
                    def pump():
                        if kk == 0:
                            st["qp"] = psQ.tile([128, QBS], F32, tag="q",
                                                name=f"qp{p}_{tb}")
                        for k in (kk, kk + 1):
                            nc.tensor.matmul(
                                st["qp"][:],
                                lhsT=wq_all[:, k, p * 128:(p + 1) * 128],
                                rhs=xa[:, k, tbc_(tb)],
                                start=(k == 0), stop=(k == NK - 1))
                        if kk == NK - 2:
                            st["raw"] = rope_pre(st["qp"])
                    return pump
                pumps = [mk(kk) for kk in range(0, NK, 2)]
                # a score-group slot sits between the drain and the rotate
                # matmul, hiding the DVE copy latency from the PE
                pumps.append(None)
                pumps.append(lambda: rope_post(
                    st["raw"], tb, [(slice(0, 128), qt_sb[p][:, tbc_(tb)])]))
                return pumps

            def kproj_pumps(tb):
                st = {}
                def mk(kk):
                    def pump():
                        if kk == 0:
                            st["kp"] = psQ.tile([128, QBS], F32, tag="q",
                                                name=f"kp{tb}")
                        for k in range(kk, kk + 4):
                            nc.tensor.matmul(
                                st["kp"][:], lhsT=wk_all[:, k, :],
                                rhs=xa[:, k, tbc_(tb)],
                                start=(k == 0), stop=(k == NK - 1))
                        if kk == NK - 4:
                            st["raw"] = rope_pre(st["kp"])
                    return pump
                pumps = [mk(kk) for kk in range(0, NK, 4)]
                pumps.append(None)
                def fin():
                    rope_post(st["raw"], tb, [
                        (slice(0, 64), ktp[(0, 0)][0:64, tbc_(tb)]),
                        (slice(64, 128), ktp[(1, 1)][64:128, tbc_(tb)]),
                    ])
                    nc.sync.dma_start(ktp[(1, 0)][64:128, tbc_(tb)],
                                      ktp[(0, 0)][0:64, tbc_(tb)])
                    nc.sync.dma_start(ktp[(0, 1)][0:64, tbc_(tb)],
                                      ktp[(1, 1)][64:128, tbc_(tb)])
                pumps.append(fin)
                return pumps

            def oproj_pumps(t, last_split=False, pool=None, pool2=None):
                st = {}
                def mk(n):
                    opool = pool if pool is not None else psO
                    if pool2 is not None and n % 2 == 1:
                        opool = pool2
                    ptag = {id(psO): "op", id(psQ): "q", id(psS): "ps"}[id(opool)]
                    def pump():
                        if n == 0:
                            st["so"] = stg.tile([128, H], BF16, tag="stg",
                                                name=f"so{t}")
                        op_ps = opool.tile([128, 512], F32, tag=ptag)
                        for p2 in range(NPAIR):
                            nc.tensor.matmul(
                                op_ps[:], lhsT=att[p2][:, t * 128:(t + 1) * 128],
                                rhs=wo_sb[p2][:, n * 512:(n + 1) * 512],
                                start=(p2 == 0), stop=(p2 == NPAIR - 1))
                        nc.vector.tensor_copy(
                            out=st["so"][:, n * 512:(n + 1) * 512], in_=op_ps[:])
                        if last_split:
                            nc.sync.dma_start(
                                out[t * 128:(t + 1) * 128,
                                    n * 512:(n + 1) * 512],
                                st["so"][:, n * 512:(n + 1) * 512])
                        elif n == 3:
                            nc.sync.dma_start(
                                out[t * 128:(t + 1) * 128, :], st["so"][:])
                    return pump
                return [mk(n) for n in range(4)]

            def interleave_nonadjacent(big, small):
                """Alternate big (chunky, shared-psum) pumps with small ones
                so consecutive big pumps never contend for their single
                PSUM buffer; pad with no-op spacers once small runs out so
                two bigs never land in the same drain slot."""
                late = 10 ** 9
                res = []
                bi, si = 0, 0
                while bi < len(big) or si < len(small):
                    if si < len(small):
                        res.append(small[si]); si += 1
                    elif bi < len(big) and res and res[-1][0] is not None:
                        res.append((None, late))
                    if bi < len(big):
                        res.append(big[bi]); bi += 1
                return res

            def build_pumps(qb):
                """Returns [(pump, deadline_sg)] for attention block qb.
                Deadlines pin same-block Q pairs before the heads that read
                them; everything else only needs to land by block end."""
                ngrp = 2 * qb + 2
                late = 10 ** 9
                qp = []
                # drains happen at the bottom of each score-group slot, so
                # a deadline of N-1 completes before slot N's score matmul
                qp += [(f, 4 * ngrp - 1) for f in qproj_pumps(2, qb)]
                if qb + 1 < NQB:
                    qp += [(f, late) for f in kproj_pumps(qb + 1)]
                qp += [(f, 6 * ngrp - 1) for f in qproj_pumps(3, qb)]
                if qb < 3:
                    qp += [(lambda t=t: vproj(t), late)
                           for t in range(4 * qb + 4, 4 * qb + 8)]
                if qb + 1 < NQB:
                    qp += [(f, late) for f in qproj_pumps(0, qb + 1)]
                    qp += [(f, late) for f in qproj_pumps(1, qb + 1)]
                og = []
                if qb == 3:
                    # no more projections in qb3, so psQ is free: alternate
                    # o-proj accumulators between psO and psQ so consecutive
                    # pumps never wait on each other's PSUM drain
                    for t in range(0, 12):
                        og += [(f, late)
                               for f in oproj_pumps(t, pool=psO, pool2=psQ)]
                    return interleave_nonadjacent(qp, og)
                return qp

            # ---------------- phase 0 ----------------
            # K proj block 0 through psS (scores have not started), V tiles
            # 0..3 alternating psS slots, first two Q pairs through psQ.
            kp0 = psS.tile([128, QBS], F32, tag="ps", name="kp0")
            for k in range(NK):
                nc.tensor.matmul(
                    kp0[:], lhsT=wk_all[:, k, :], rhs=xa[:, k, tbc_(0)],
                    start=(k == 0), stop=(k == NK - 1))
            kfinish(kp0, 0)
            for t in range(4):
                vproj(t, pool=psS, tag="ps")
            for pump in qproj_pumps(0, 0) + qproj_pumps(1, 0):
                if pump is not None:
                    pump()

            # ---------------- attention ----------------
            for qb in range(NQB):
                pumps = build_pumps(qb)
                # suffix-min of deadlines so a due pump forces everything
                # queued in front of it out as well
                due = [0] * (len(pumps) + 1)
                due[len(pumps)] = 10 ** 9
                for i in range(len(pumps) - 1, -1, -1):
                    due[i] = min(pumps[i][1], due[i + 1])
                n_sg = QH * (2 * qb + 2)
                pi = 0
                attq_cur = None
                for hh in range(QH):
                    p = hh // 2
                    half = hh % 2
                    v = hh // 4
                    ksel = ktp[(half, v)]
                    nkv = 4 * qb + 4
                    ngrp = nkv // 2
                    if half == 0:
                        attq_cur = [aq.tile([128, 128], BF16, tag="attq",
                                            name=f"attq{qb}_{p}_{s}")
                                    for s in range(4)]
                    ov = psV.tile([128, 4, D + 1], F32, tag="ov")
                    pts = []
                    for g2 in range(ngrp):
                        sc = psS.tile([128, 2 * QBS], F32, tag="ps")
                        pt = wk.tile([128, 2 * QBS], BF16, tag="pt")
                        j0 = 2 * g2 - 4 * qb
                        for ht in (0, 1):
                            kv = 2 * g2 + ht
                            j = kv - 4 * qb
                            c0 = 128 * j if j > 0 else 0
                            base = ht * QBS
                            nc.tensor.matmul(
                                sc[:, base + c0:base + QBS],
                                lhsT=ksel[:, kv * 128:(kv + 1) * 128],
                                rhs=qt_sb[p][:, qb * QBS + c0:(qb + 1) * QBS],
                                start=True, stop=True)
                        if j0 + 1 < 0:
                            nc.scalar.activation(pt[:], sc[:], AF.Exp)
                        else:
                            for ht in (0, 1):
                                j = 2 * g2 + ht - 4 * qb
                                c0 = 128 * j if j > 0 else 0
                                base = ht * QBS
                                nc.scalar.activation(
                                    pt[:, base + c0:base + QBS],
                                    sc[:, base + c0:base + QBS], AF.Exp)
                        for ht in (0, 1):
                            j = 2 * g2 + ht - 4 * qb
                            if j >= 0:
                                c0 = 128 * j
                                base = ht * QBS
                                nc.vector.tensor_tensor(
                                    out=pt[:, base + c0:base + c0 + 128],
                                    in0=pt[:, base + c0:base + c0 + 128],
                                    in1=tri[:], op=OP.mult)
                        pts.append(pt)
                        if g2 > 0:
                            _pv_group(nc, ov, pts[g2 - 1], va[v], qb, g2 - 1, nkv)
                        # self-correcting pacing: spread the remaining pumps
                        # evenly over the remaining score-group slots, and
                        # force any whose deadline is due
                        sg_idx = hh * ngrp + g2
                        sgs_left = n_sg - sg_idx
                        want = -((pi - len(pumps)) // sgs_left)  # ceil div
                        emitted = 0
                        while pi < len(pumps) and (
                                emitted < want or due[pi] <= sg_idx):
                            if pumps[pi][0] is not None:
                                pumps[pi][0]()
                            pi += 1
                            emitted += 1
                    _pv_group(nc, ov, pts[ngrp - 1], va[v], qb, ngrp - 1, nkv)
                    # normalize: denominator sits at free position 64
                    rec = wk2.tile([128, 4], F32, tag="rec")
                    nc.vector.reciprocal(rec[:, :], ov[:, :, D])
                    for s in range(4):
                        nc.vector.tensor_scalar(
                            out=attq_cur[s][:, half * D:(half + 1) * D],
                            in0=ov[:, s, 0:D],
                            scalar1=rec[:, s:s + 1], scalar2=None, op0=OP.mult)
                    if half == 1:
                        for s in range(4):
                            nc.sync.dma_start_transpose(
                                att[p][:, (qb * 4 + s) * 128:(qb * 4 + s + 1) * 128],
                                attq_cur[s][:])
                while pi < len(pumps):
                    if pumps[pi][0] is not None:
                        pumps[pi][0]()
                    pi += 1
            # trailing o-proj: scores are done, so the psS banks are free
            # and give the accumulators double-buffering; stream the output
            # DMAs per half tile so the final drain is short
            for t in range(12, 16):
                for pump in oproj_pumps(t, last_split=True, pool=psS):
                    pump()
    nc.compile()
    return nc


def _pv_group(nc, ov, pt, vat, qb, g2, nkv):
    """Accumulate PV matmuls for score group g2 (kv tiles 2*g2, 2*g2+1)."""
    for ht in (0, 1):
        kv = 2 * g2 + ht
        j = kv - 4 * qb
        for s in range(4):
            if j > s:
                continue  # this kv tile is fully masked for q subtile s
            nc.tensor.matmul(
                ov[:, s, :],
                lhsT=pt[:, ht * QBS + s * 128:ht * QBS + (s + 1) * 128],
                rhs=vat[:, kv, :],
                start=(kv == 0 and s == 0),
                stop=(kv == nkv - 1 and s == 3))


def _host_tables():
    freq = 1.0 / (10000.0 ** (np.arange(0, D, 2, dtype=np.float64) / D))
    t = np.arange(S, dtype=np.float64)
    fr = t[:, None] * freq[None, :]                       # (S, 32)
    emb = np.concatenate([fr, fr], axis=-1)               # (S, 64)
    cos64 = np.cos(emb).T.astype(np.float32)              # (64, S)
    sin64 = np.sin(emb).T.astype(np.float32)
    cos128 = np.concatenate([cos64, cos64], axis=0).astype(BF)
    sin128 = np.concatenate([sin64, sin64], axis=0).astype(BF)
    R = np.zeros((64, 64), np.float32)
    R[np.arange(32), 32 + np.arange(32)] = -1.0
    R[32 + np.arange(32), np.arange(32)] = 1.0
    R128 = np.zeros((128, 128), np.float32)
    R128[:64, :64] = R
    R128[64:, 64:] = R
    rotT = np.ascontiguousarray(R128.T).astype(BF)
    r = np.arange(128)[:, None]
    c = np.arange(128)[None, :]
    trimask = (r <= c).astype(np.float32).astype(BF)
    return cos128, sin128, rotT, trimask


def _make_in_maps(inputs):
    x = np.asarray(inputs["x"], np.float32)
    Wq = np.asarray(inputs["Wq"], np.float32)
    Wk = np.asarray(inputs["Wk"], np.float32)
    Wv = np.asarray(inputs["Wv"], np.float32)
    Wo = np.asarray(inputs["Wo"], np.float32)
    cos128, sin128, rotT, trimask = _host_tables()
    in_maps = []
    for core in range(8):
        g, b = core // 2, core % 2
        im = {
            "xT": np.ascontiguousarray(x[b].T).astype(BF),
            "wqT": np.ascontiguousarray((Wq[QF * g:QF * (g + 1), :] / 8.0).T).astype(BF),
            "wkT": np.ascontiguousarray(Wk[KF * g:KF * (g + 1), :].T).astype(BF),
            "wvT": np.ascontiguousarray(Wv[KF * g:KF * (g + 1), :].T).astype(BF),
            "woT": np.ascontiguousarray(Wo[:, QF * g:QF * (g + 1)].T).astype(BF),
            "cost": cos128,
            "sint": sin128,
            "rotT": rotT,
            "trim": trimask,
        }
        in_maps.append(im)
    return in_maps


def kernel(x, Wq, Wk, Wv, Wo):
    if "nc" not in _CACHE:
        _CACHE["nc"] = _build_program()
    nc = _CACHE["nc"]

    in_maps = _make_in_maps(
        {"x": x, "Wq": Wq, "Wk": Wk, "Wv": Wv, "Wo": Wo})

    trace = bool(int(os.environ.get("KERNEL_TRACE", "0")))
    res = bass_utils.run_bass_kernel_spmd(
        nc, in_maps, core_ids=list(range(8)), trace=trace)
    _CACHE["last_result"] = res

    out = np.zeros((B, S, H), np.float32)
    for core in range(8):
        g, b = core // 2, core % 2
        out[b] += np.asarray(res.results[core]["out"], np.float32)
    return out


# revision 48
# speedup vs baseline: 1.0529x; 1.0096x over previous
"""GQA attention kernel for Trainium2 (8 NeuronCores).

Sharding: batch x head-group tensor parallel. Core c handles batch (c % 2)
and head group (c // 2): 8 q heads + 2 kv heads of that batch. Each core
computes its partial o-proj output (contraction over its 512 attn features);
the host sums the 4 partials per batch.

Device-side dataflow (per core):
  xT   [H=2048 hidden, S=2048 tokens] bf16  (x transposed on host)
  Q^T  [dim, tokens] per head-pair tile [128, S]   (projection with W
       stationary), RoPE'd via a fixed 128x128 rotation matmul + cos/sin.
  K^T  same, zero-padded into 4 [128, S] variants keyed (side, kv-head)
       so score matmuls can contract over the full 128 partitions.
  V    [tokens, dim] natural layout with an appended ones-column.
  scores S^T[kv, q] = K^T.T @ Q^T computed in 2-kv-tile groups into a
       bank-spanning PSUM tile [128, 1024]; exp'd with one ACT instr per
       group; only the diagonal 128x128 sub-block is masked (DVE mult by
       a fixed triangular 0/1 mask).
  PV   out[q, d+1] = pt.T @ V  (probs stationary, q tokens on PSUM
       partitions, streams only 65 columns) accumulated over kv tiles in
       a single PSUM accumulation group per (q-block, head). The ones
       column lands the softmax denominator at free position 64, so
       normalization is a per-partition reciprocal + tensor_scalar.
  attn^T for o-proj is rebuilt via XBAR DMA transposes (SBUF->SBUF) of
       the normalized [q, feature-pair] tiles.
  o-proj streams Wo with attn^T stationary; PSUM results staged to SBUF
       and DMA'd out per 128-token row. The 1/sqrt(64) scale is folded
       into Wq on the host.

The attention loop is ACT(exp)-throughput-bound, so ALL other PE work
(Q/K/V projections, o-proj) is chopped into sub-microsecond "pumps" and
woven between score groups to fill the PE stall slots while ACT drains
the exp backlog. Attention q-block qb only needs K/V token tiles up to
4*qb+3, so K proj of block tb feeds attention block tb as a pump one
block ahead.
"""

import os
import numpy as np
import ml_dtypes
from contextlib import ExitStack

import concourse.bass as bass
import concourse.tile as tile
from concourse import bacc
from concourse import mybir
from concourse import bass_utils

BF16 = mybir.dt.bfloat16
F32 = mybir.dt.float32
BF = ml_dtypes.bfloat16
AF = mybir.ActivationFunctionType
OP = mybir.AluOpType

H = 2048
S = 2048
B = 2
D = 64
QH = 8            # q heads per core
KVH = 2           # kv heads per core
QF = QH * D       # 512 q features per core
KF = KVH * D      # 128 kv features per core
NK = H // 128     # 16 contraction tiles
NT = S // 128     # 16 token tiles
QBS = 512         # q block size
NQB = S // QBS    # 4 q blocks
NPAIR = QF // 128 # 4 q head-pair tiles

_CACHE = {}


def tbc_(tb):
    return slice(tb * QBS, (tb + 1) * QBS)


def _build_program():
    nc = bacc.Bacc(
        "TRN2",
        target_bir_lowering=False,
        debug=False,
        enable_asserts=False,
        num_devices=8,
    )
    xT = nc.dram_tensor("xT", [H, S], BF16, kind="ExternalInput").ap()
    wqT = nc.dram_tensor("wqT", [H, QF], BF16, kind="ExternalInput").ap()
    wkT = nc.dram_tensor("wkT", [H, KF], BF16, kind="ExternalInput").ap()
    wvT = nc.dram_tensor("wvT", [H, KF], BF16, kind="ExternalInput").ap()
    woT = nc.dram_tensor("woT", [QF, H], BF16, kind="ExternalInput").ap()
    cost = nc.dram_tensor("cost", [128, S], BF16, kind="ExternalInput").ap()
    sint = nc.dram_tensor("sint", [128, S], BF16, kind="ExternalInput").ap()
    rotT = nc.dram_tensor("rotT", [128, 128], BF16, kind="ExternalInput").ap()
    trim = nc.dram_tensor("trim", [128, 128], BF16, kind="ExternalInput").ap()
    out = nc.dram_tensor("out", [S, H], BF16, kind="ExternalOutput").ap()

    with tile.TileContext(nc) as tc:
        with ExitStack() as ctx:
            E = ctx.enter_context
            persist = E(tc.tile_pool(name="persist", bufs=1))
            psS = E(tc.tile_pool(name="psS", bufs=2, space="PSUM"))
            psQ = E(tc.tile_pool(name="psQ", bufs=1, space="PSUM"))
            psV = E(tc.tile_pool(name="psV", bufs=2, space="PSUM"))
            psO = E(tc.tile_pool(name="psO", bufs=1, space="PSUM"))
            wk = E(tc.tile_pool(name="wk", bufs=4))
            wk2 = E(tc.tile_pool(name="wk2", bufs=2))
            aq = E(tc.tile_pool(name="aq", bufs=8))
            stg = E(tc.tile_pool(name="stg", bufs=3))

            # ---------------- batched loads ----------------
            # DMA device is the startup bottleneck; order/chunk loads so
            # K proj / V proj / first Q projs start as early as possible.
            wk_all = persist.tile([128, NK, KF], BF16, tag="wk_all")
            wkr = wkT.rearrange("(k p) c -> p k c", p=128)
            nc.sync.dma_start(wk_all[:, 0:8, :], wkr[:, 0:8, :])
            xa = persist.tile([128, NK, S], BF16, tag="xa")
            xr = xT.rearrange("(k p) s -> p k s", p=128)
            nc.sync.dma_start(xa[:, 0:4, tbc_(0)], xr[:, 0:4, tbc_(0)])
            nc.sync.dma_start(wk_all[:, 8:16, :], wkr[:, 8:16, :])
            for kc in range(4, NK, 4):
                nc.sync.dma_start(
                    xa[:, kc:kc + 4, tbc_(0)], xr[:, kc:kc + 4, tbc_(0)])
            wv_all = persist.tile([128, NK, KF], BF16, tag="wv_all")
            nc.sync.dma_start(
                wv_all[:], wvT.rearrange("(k p) c -> p k c", p=128))
            rt = persist.tile([128, 128], BF16, tag="rt")
            nc.sync.dma_start(rt[:], rotT[:, :])
            cs = persist.tile([128, S], BF16, tag="cs")
            nc.sync.dma_start(cs[:], cost[:, :])
            sn = persist.tile([128, S], BF16, tag="sn")
            nc.sync.dma_start(sn[:], sint[:, :])
            wq_all = persist.tile([128, NK, QF], BF16, tag="wq_all")
            wqr = wqT.rearrange("(k p) c -> p k c", p=128)
            nc.sync.dma_start(wq_all[:, :, 0:256], wqr[:, :, 0:256])
            tri = persist.tile([128, 128], BF16, tag="tri")
            nc.sync.dma_start(tri[:], trim[:, :])
            nc.sync.dma_start(wq_all[:, :, 256:512], wqr[:, :, 256:512])
            for tb in range(1, NQB):
                nc.sync.dma_start(xa[:, :, tbc_(tb)], xr[:, :, tbc_(tb)])
            wo_sb = []
            for p in range(NPAIR):
                t = persist.tile([128, H], BF16, tag=f"wo{p}", name=f"wo{p}")
                nc.sync.dma_start(t[:], woT[p * 128:(p + 1) * 128, :])
                wo_sb.append(t)

            # ---------------- persistent activation tiles ----------------
            qt_sb = [persist.tile([128, S], BF16, tag=f"qt{p}", name=f"qt{p}") for p in range(NPAIR)]
            ktp = {(sd, v): persist.tile([128, S], BF16, tag=f"ktp{sd}{v}", name=f"ktp{sd}{v}")
                   for sd in (0, 1) for v in (0, 1)}
            va = [persist.tile([128, NT, D + 1], BF16, tag=f"va{v}", name=f"va{v}") for v in (0, 1)]
            att = [persist.tile([128, S], BF16, tag=f"att{p}", name=f"att{p}") for p in range(NPAIR)]

            # zero pads (Pool engine; it is otherwise idle, and these have
            # no input dependencies so they run during the initial loads)
            nc.gpsimd.memset(ktp[(0, 0)][64:128, :], 0.0)
            nc.gpsimd.memset(ktp[(1, 1)][0:64, :], 0.0)
            nc.gpsimd.memset(ktp[(1, 0)][0:64, :], 0.0)
            nc.gpsimd.memset(ktp[(0, 1)][64:128, :], 0.0)
            nc.gpsimd.memset(va[0][:, :, D:D + 1], 1.0)
            nc.gpsimd.memset(va[1][:, :, D:D + 1], 1.0)

            def rope_pre(ps):
                """Drain the projection accumulator to SBUF (releases the
                psQ bank); the rest of RoPE runs in rope_post."""
                raw = wk.tile([128, QBS], BF16, tag="rope_raw")
                nc.vector.tensor_copy(out=raw[:], in_=ps[:])
                return raw

            def rope_post(raw, tb, outs, pool=None, tag="q"):
                rp = (pool or psQ).tile([128, QBS], F32, tag=tag)
                nc.tensor.matmul(rp[:], lhsT=rt[:], rhs=raw[:], start=True, stop=True)
                t1 = wk.tile([128, QBS], BF16, tag="rope_t1")
                nc.vector.tensor_tensor(out=t1[:], in0=rp[:], in1=sn[:, tbc_(tb)], op=OP.mult)
                t2 = wk.tile([128, QBS], BF16, tag="rope_t2")
                nc.vector.tensor_tensor(out=t2[:], in0=raw[:], in1=cs[:, tbc_(tb)], op=OP.mult)
                for rows, out_ap in outs:
                    nc.vector.tensor_tensor(
                        out=out_ap, in0=t1[rows, :], in1=t2[rows, :], op=OP.add)

            def rope(ps, tb, outs):
                rope_post(rope_pre(ps), tb, outs)

            def vproj(t, pool=None, tag="q"):
                vp = (pool or psQ).tile([128, KF], F32, tag=tag)
                for k in range(NK):
                    nc.tensor.matmul(
                        vp[:], lhsT=xa[:, k, t * 128:(t + 1) * 128],
                        rhs=wv_all[:, k, :],
                        start=(k == 0), stop=(k == NK - 1))
                for v in (0, 1):
                    nc.vector.tensor_copy(
                        out=va[v][:, t, 0:D], in_=vp[:, v * D:(v + 1) * D])

            # ---------------- pump machinery ----------------
            # Each pump is a closure emitting ~0.2-0.9us of PE work; pumps
            # are interleaved between attention score groups. All
            # projection PSUM flows through the single psQ bank: within a
            # generator the accumulator's reader (rope raw copy / va copy)
            # is emitted before the next psQ allocation, so the pool
            # rotation never stalls.
            def qproj_pumps(p, tb, pool=None, tag="q"):
                pool = pool or psQ
                st = {}
                def mk(kk):
                    def pump():
                        if kk == 0:
                            st["qp"] = pool.tile([128, QBS], F32, tag=tag,
                                                 name=f"qp{p}_{tb}")
                        for k in (kk, kk + 1):
                            nc.tensor.matmul(
                                st["qp"][:],
                                lhsT=wq_all[:, k, p * 128:(p + 1) * 128],
                                rhs=xa[:, k, tbc_(tb)],
                                start=(k == 0), stop=(k == NK - 1))
                        if kk == NK - 2:
                            st["raw"] = rope_pre(st["qp"])
                    return pump
                pumps = [mk(kk) for kk in range(0, NK, 2)]
                # a score-group slot sits between the drain and the rotate
                # matmul, hiding the DVE copy latency from the PE
                pumps.append(None)
                pumps.append(lambda: rope_post(
                    st["raw"], tb, [(slice(0, 128), qt_sb[p][:, tbc_(tb)])],
                    pool=pool, tag=tag))
                return pumps

            def kproj_pumps(tb, pool=None, tag="q"):
                pool = pool or psQ
                st = {}
                def mk(kk):
                    def pump():
                        if kk == 0:
                            st["kp"] = pool.tile([128, QBS], F32, tag=tag,
                                                 name=f"kp{tb}")
                        for k in range(kk, kk + 4):
                            nc.tensor.matmul(
                                st["kp"][:], lhsT=wk_all[:, k, :],
                                rhs=xa[:, k, tbc_(tb)],
                                start=(k == 0), stop=(k == NK - 1))
                        if kk == NK - 4:
                            st["raw"] = rope_pre(st["kp"])
                    return pump
                pumps = [mk(kk) for kk in range(0, NK, 4)]
                pumps.append(None)
                def fin():
                    rope_post(st["raw"], tb, [
                        (slice(0, 64), ktp[(0, 0)][0:64, tbc_(tb)]),
                        (slice(64, 128), ktp[(1, 1)][64:128, tbc_(tb)]),
                    ], pool=pool, tag=tag)
                    nc.sync.dma_start(ktp[(1, 0)][64:128, tbc_(tb)],
                                      ktp[(0, 0)][0:64, tbc_(tb)])
                    nc.sync.dma_start(ktp[(0, 1)][0:64, tbc_(tb)],
                                      ktp[(1, 1)][64:128, tbc_(tb)])
                pumps.append(fin)
                return pumps

            def oproj_pumps(t, last_split=False, pool=None, pool2=None):
                st = {}
                def mk(n):
                    opool = pool if pool is not None else psO
                    if pool2 is not None and n % 2 == 1:
                        opool = pool2
                    ptag = {id(psO): "op", id(psQ): "q", id(psS): "ps"}[id(opool)]
                    def pump():
                        if n == 0:
                            st["so"] = stg.tile([128, H], BF16, tag="stg",
                                                name=f"so{t}")
                        op_ps = opool.tile([128, 512], F32, tag=ptag)
                        for p2 in range(NPAIR):
                            nc.tensor.matmul(
                                op_ps[:], lhsT=att[p2][:, t * 128:(t + 1) * 128],
                                rhs=wo_sb[p2][:, n * 512:(n + 1) * 512],
                                start=(p2 == 0), stop=(p2 == NPAIR - 1))
                        nc.vector.tensor_copy(
                            out=st["so"][:, n * 512:(n + 1) * 512], in_=op_ps[:])
                        if last_split:
                            nc.sync.dma_start(
                                out[t * 128:(t + 1) * 128,
                                    n * 512:(n + 1) * 512],
                                st["so"][:, n * 512:(n + 1) * 512])
                        elif n == 3:
                            nc.sync.dma_start(
                                out[t * 128:(t + 1) * 128, :], st["so"][:])
                    return pump
                return [mk(n) for n in range(4)]

            def interleave_nonadjacent(big, small):
                """Alternate big (chunky, shared-psum) pumps with small ones
                so consecutive big pumps never contend for their single
                PSUM buffer; pad with no-op spacers once small runs out so
                two bigs never land in the same drain slot."""
                late = 10 ** 9
                res = []
                bi, si = 0, 0
                while bi < len(big) or si < len(small):
                    if si < len(small):
                        res.append(small[si]); si += 1
                    elif bi < len(big) and res and res[-1][0] is not None:
                        res.append((None, late))
                    if bi < len(big):
                        res.append(big[bi]); bi += 1
                return res

            def build_pumps(qb):
                """Returns [(pump, deadline_sg)] for attention block qb.
                Deadlines pin same-block Q pairs before the heads that read
                them; everything else only needs to land by block end."""
                ngrp = 2 * qb + 2
                late = 10 ** 9
                qp = []
                # drains happen at the bottom of each score-group slot, so
                # a deadline of N-1 completes before slot N's score matmul.
                # Generators are joined with spacer slots so the psQ hand-off
                # latency (DVE drain + sem) hides behind a score group.
                gens = []
                if qb == 0:
                    gens.append([(f, 2 * ngrp - 1)
                                 for f in qproj_pumps(1, 0)])
                gens.append([(f, 4 * ngrp - 1) for f in qproj_pumps(2, qb)])
                if qb + 1 < NQB:
                    gens.append([(f, late) for f in kproj_pumps(qb + 1)])
                gens.append([(f, 6 * ngrp - 1) for f in qproj_pumps(3, qb)])
                if qb < 3:
                    vg = []
                    for t in range(4 * qb + 4, 4 * qb + 8):
                        if vg:
                            vg.append((None, late))
                        vg.append((lambda t=t: vproj(t), late))
                    gens.append(vg)
                if qb + 1 < NQB:
                    gens.append([(f, late) for f in qproj_pumps(0, qb + 1)])
                    gens.append([(f, late) for f in qproj_pumps(1, qb + 1)])
                for g in gens:
                    if qp:
                        qp.append((None, late))
                    qp += g
                og = []
                if qb == 3:
                    # no more projections in qb3, so psQ is free: alternate
                    # o-proj accumulators between psO and psQ so consecutive
                    # pumps never wait on each other's PSUM drain
                    for t in range(0, 12):
                        og += [(f, late)
                               for f in oproj_pumps(t, pool=psO, pool2=psQ)]
                    return interleave_nonadjacent(qp, og)
                return qp

            # ---------------- phase 0 ----------------
            # K proj block 0 through psS (scores have not started), V tiles
            # 0..3 alternating psS slots while the cos/sin tables are still
            # loading, then K RoPE and the first two Q pairs through psQ.
            kp0 = psS.tile([128, QBS], F32, tag="ps", name="kp0")
            for k in range(NK):
                nc.tensor.matmul(
                    kp0[:], lhsT=wk_all[:, k, :], rhs=xa[:, k, tbc_(0)],
                    start=(k == 0), stop=(k == NK - 1))
            raw0 = rope_pre(kp0)
            for t in range(4):
                vproj(t, pool=psS, tag="ps")
            rope_post(raw0, 0, [
                (slice(0, 64), ktp[(0, 0)][0:64, tbc_(0)]),
                (slice(64, 128), ktp[(1, 1)][64:128, tbc_(0)]),
            ])
            nc.sync.dma_start(ktp[(1, 0)][64:128, tbc_(0)],
                              ktp[(0, 0)][0:64, tbc_(0)])
            nc.sync.dma_start(ktp[(0, 1)][0:64, tbc_(0)],
                              ktp[(1, 1)][64:128, tbc_(0)])
            for pump in qproj_pumps(0, 0):
                if pump is not None:
                    pump()

            # ---------------- attention ----------------
            for qb in range(NQB):
                pumps = build_pumps(qb)
                # suffix-min of deadlines so a due pump forces everything
                # queued in front of it out as well
                due = [0] * (len(pumps) + 1)
                due[len(pumps)] = 10 ** 9
                for i in range(len(pumps) - 1, -1, -1):
                    due[i] = min(pumps[i][1], due[i + 1])
                n_sg = QH * (2 * qb + 2)
                pi = 0
                attq_cur = None
                for hh in range(QH):
                    p = hh // 2
                    half = hh % 2
                    v = hh // 4
                    ksel = ktp[(half, v)]
                    nkv = 4 * qb + 4
                    ngrp = nkv // 2
                    if half == 0:
                        attq_cur = [aq.tile([128, 128], BF16, tag="attq",
                                            name=f"attq{qb}_{p}_{s}")
                                    for s in range(4)]
                    ov = psV.tile([128, 4, D + 1], F32, tag="ov")
                    pts = []
                    for g2 in range(ngrp):
                        sc = psS.tile([128, 2 * QBS], F32, tag="ps")
                        pt = wk.tile([128, 2 * QBS], BF16, tag="pt")
                        j0 = 2 * g2 - 4 * qb
                        for ht in (0, 1):
                            kv = 2 * g2 + ht
                            j = kv - 4 * qb
                            c0 = 128 * j if j > 0 else 0
                            base = ht * QBS
                            nc.tensor.matmul(
                                sc[:, base + c0:base + QBS],
                                lhsT=ksel[:, kv * 128:(kv + 1) * 128],
                                rhs=qt_sb[p][:, qb * QBS + c0:(qb + 1) * QBS],
                                start=True, stop=True)
                        if j0 + 1 < 0:
                            nc.scalar.activation(pt[:], sc[:], AF.Exp)
                        else:
                            for ht in (0, 1):
                                j = 2 * g2 + ht - 4 * qb
                                c0 = 128 * j if j > 0 else 0
                                base = ht * QBS
                                nc.scalar.activation(
                                    pt[:, base + c0:base + QBS],
                                    sc[:, base + c0:base + QBS], AF.Exp)
                        for ht in (0, 1):
                            j = 2 * g2 + ht - 4 * qb
                            if j >= 0:
                                c0 = 128 * j
                                base = ht * QBS
                                nc.vector.tensor_tensor(
                                    out=pt[:, base + c0:base + c0 + 128],
                                    in0=pt[:, base + c0:base + c0 + 128],
                                    in1=tri[:], op=OP.mult)
                        pts.append(pt)
                        if g2 > 0:
                            _pv_group(nc, ov, pts[g2 - 1], va[v], qb, g2 - 1, nkv)
                        # self-correcting pacing: spread the remaining pumps
                        # evenly over the remaining score-group slots, and
                        # force any whose deadline is due
                        sg_idx = hh * ngrp + g2
                        sgs_left = n_sg - sg_idx
                        want = -((pi - len(pumps)) // sgs_left)  # ceil div
                        emitted = 0
                        while pi < len(pumps) and (
                                emitted < want or due[pi] <= sg_idx):
                            if pumps[pi][0] is not None:
                                pumps[pi][0]()
                            pi += 1
                            emitted += 1
                    _pv_group(nc, ov, pts[ngrp - 1], va[v], qb, ngrp - 1, nkv)
                    # normalize: denominator sits at free position 64
                    rec = wk2.tile([128, 4], F32, tag="rec")
                    nc.vector.reciprocal(rec[:, :], ov[:, :, D])
                    for s in range(4):
                        nc.vector.tensor_scalar(
                            out=attq_cur[s][:, half * D:(half + 1) * D],
                            in0=ov[:, s, 0:D],
                            scalar1=rec[:, s:s + 1], scalar2=None, op0=OP.mult)
                    if half == 1:
                        for s in range(4):
                            nc.sync.dma_start_transpose(
                                att[p][:, (qb * 4 + s) * 128:(qb * 4 + s + 1) * 128],
                                attq_cur[s][:])
                while pi < len(pumps):
                    if pumps[pi][0] is not None:
                        pumps[pi][0]()
                    pi += 1
            # trailing o-proj: scores are done, so the psS banks are free
            # and give the accumulators double-buffering; stream the output
            # DMAs per half tile so the final drain is short
            for t in range(12, 16):
                for pump in oproj_pumps(t, last_split=True, pool=psS):
                    pump()
    nc.compile()
    return nc


def _pv_group(nc, ov, pt, vat, qb, g2, nkv):
    """Accumulate PV matmuls for score group g2 (kv tiles 2*g2, 2*g2+1)."""
    for ht in (0, 1):
        kv = 2 * g2 + ht
        j = kv - 4 * qb
        for s in range(4):
            if j > s:
                continue  # this kv tile is fully masked for q subtile s
            nc.tensor.matmul(
                ov[:, s, :],
                lhsT=pt[:, ht * QBS + s * 128:ht * QBS + (s + 1) * 128],
                rhs=vat[:, kv, :],
                start=(kv == 0 and s == 0),
                stop=(kv == nkv - 1 and s == 3))


def _host_tables():
    freq = 1.0 / (10000.0 ** (np.arange(0, D, 2, dtype=np.float64) / D))
    t = np.arange(S, dtype=np.float64)
    fr = t[:, None] * freq[None, :]                       # (S, 32)
    emb = np.concatenate([fr, fr], axis=-1)               # (S, 64)
    cos64 = np.cos(emb).T.astype(np.float32)              # (64, S)
    sin64 = np.sin(emb).T.astype(np.float32)
    cos128 = np.concatenate([cos64, cos64], axis=0).astype(BF)
    sin128 = np.concatenate([sin64, sin64], axis=0).astype(BF)
    R = np.zeros((64, 64), np.float32)
    R[np.arange(32), 32 + np.arange(32)] = -1.0
    R[32 + np.arange(32), np.arange(32)] = 1.0
    R128 = np.zeros((128, 128), np.float32)
    R128[:64, :64] = R
    R128[64:, 64:] = R
    rotT = np.ascontiguousarray(R128.T).astype(BF)
    r = np.arange(128)[:, None]
    c = np.arange(128)[None, :]
    trimask = (r <= c).astype(np.float32).astype(BF)
    return cos128, sin128, rotT, trimask


def _make_in_maps(inputs):
    x = np.asarray(inputs["x"], np.float32)
    Wq = np.asarray(inputs["Wq"], np.float32)
    Wk = np.asarray(inputs["Wk"], np.float32)
    Wv = np.asarray(inputs["Wv"], np.float32)
    Wo = np.asarray(inputs["Wo"], np.float32)
    cos128, sin128, rotT, trimask = _host_tables()
    in_maps = []
    for core in range(8):
        g, b = core // 2, core % 2
        im = {
            "xT": np.ascontiguousarray(x[b].T).astype(BF),
            "wqT": np.ascontiguousarray((Wq[QF * g:QF * (g + 1), :] / 8.0).T).astype(BF),
            "wkT": np.ascontiguousarray(Wk[KF * g:KF * (g + 1), :].T).astype(BF),
            "wvT": np.ascontiguousarray(Wv[KF * g:KF * (g + 1), :].T).astype(BF),
            "woT": np.ascontiguousarray(Wo[:, QF * g:QF * (g + 1)].T).astype(BF),
            "cost": cos128,
            "sint": sin128,
            "rotT": rotT,
            "trim": trimask,
        }
        in_maps.append(im)
    return in_maps


def kernel(x, Wq, Wk, Wv, Wo):
    if "nc" not in _CACHE:
        _CACHE["nc"] = _build_program()
    nc = _CACHE["nc"]

    in_maps = _make_in_maps(
        {"x": x, "Wq": Wq, "Wk": Wk, "Wv": Wv, "Wo": Wo})

    trace = bool(int(os.environ.get("KERNEL_TRACE", "0")))
    res = bass_utils.run_bass_kernel_spmd(
        nc, in_maps, core_ids=list(range(8)), trace=trace)
    _CACHE["last_result"] = res

    out = np.zeros((B, S, H), np.float32)
    for core in range(8):
        g, b = core // 2, core % 2
        out[b] += np.asarray(res.results[core]["out"], np.float32)
    return out


# revision 49
# speedup vs baseline: 1.1162x; 1.0601x over previous
"""GQA attention kernel for Trainium2 (8 NeuronCores).

Sharding: batch x head-group tensor parallel. Core c handles batch (c % 2)
and head group (c // 2): 8 q heads + 2 kv heads of that batch. Each core
computes its partial o-proj output (contraction over its 512 attn features);
the host sums the 4 partials per batch.

Device-side dataflow (per core):
  xT   [H=2048 hidden, S=2048 tokens] bf16  (x transposed on host)
  Q^T  [dim, tokens] per head-pair tile [128, S]   (projection with W
       stationary), RoPE'd via a fixed 128x128 rotation matmul + cos/sin.
  K^T  same, zero-padded into 4 [128, S] variants keyed (side, kv-head)
       so score matmuls can contract over the full 128 partitions.
  V    [tokens, dim] natural layout with an appended ones-column.
  scores S^T[kv, q] = K^T.T @ Q^T computed in 2-kv-tile groups into a
       bank-spanning PSUM tile [128, 1024]; exp'd with one ACT instr per
       group; only the diagonal 128x128 sub-block is masked (DVE mult by
       a fixed triangular 0/1 mask).
  PV   out[q, d+1] = pt.T @ V  (probs stationary, q tokens on PSUM
       partitions, streams only 65 columns) accumulated over kv tiles in
       a single PSUM accumulation group per (q-block, head). The ones
       column lands the softmax denominator at free position 64, so
       normalization is a per-partition reciprocal + tensor_scalar.
  attn^T for o-proj is rebuilt via XBAR DMA transposes (SBUF->SBUF) of
       the normalized [q, feature-pair] tiles.
  o-proj streams Wo with attn^T stationary; PSUM results staged to SBUF
       and DMA'd out per 128-token row. The 1/sqrt(64) scale is folded
       into Wq on the host.

The attention loop is ACT(exp)-throughput-bound, so ALL other PE work
(Q/K/V projections, o-proj) is chopped into sub-microsecond "pumps" and
woven between score groups to fill the PE stall slots while ACT drains
the exp backlog. Attention q-block qb only needs K/V token tiles up to
4*qb+3, so K proj of block tb feeds attention block tb as a pump one
block ahead.
"""

import os
import numpy as np
import ml_dtypes
from contextlib import ExitStack

import concourse.bass as bass
import concourse.tile as tile
from concourse import bacc
from concourse import mybir
from concourse import bass_utils

BF16 = mybir.dt.bfloat16
F32 = mybir.dt.float32
BF = ml_dtypes.bfloat16
AF = mybir.ActivationFunctionType
OP = mybir.AluOpType

H = 2048
S = 2048
B = 2
D = 64
QH = 8            # q heads per core
KVH = 2           # kv heads per core
QF = QH * D       # 512 q features per core
KF = KVH * D      # 128 kv features per core
NK = H // 128     # 16 contraction tiles
NT = S // 128     # 16 token tiles
QBS = 512         # q block size
NQB = S // QBS    # 4 q blocks
NPAIR = QF // 128 # 4 q head-pair tiles

_CACHE = {}


def tbc_(tb):
    return slice(tb * QBS, (tb + 1) * QBS)


def _build_program():
    nc = bacc.Bacc(
        "TRN2",
        target_bir_lowering=False,
        debug=False,
        enable_asserts=False,
        num_devices=8,
    )
    xT = nc.dram_tensor("xT", [H, S], BF16, kind="ExternalInput").ap()
    wqT = nc.dram_tensor("wqT", [H, QF], BF16, kind="ExternalInput").ap()
    wkT = nc.dram_tensor("wkT", [H, KF], BF16, kind="ExternalInput").ap()
    wvT = nc.dram_tensor("wvT", [H, KF], BF16, kind="ExternalInput").ap()
    woT = nc.dram_tensor("woT", [QF, H], BF16, kind="ExternalInput").ap()
    cost = nc.dram_tensor("cost", [128, S], BF16, kind="ExternalInput").ap()
    sint = nc.dram_tensor("sint", [128, S], BF16, kind="ExternalInput").ap()
    rotT = nc.dram_tensor("rotT", [128, 128], BF16, kind="ExternalInput").ap()
    trim = nc.dram_tensor("trim", [128, 128], BF16, kind="ExternalInput").ap()
    out = nc.dram_tensor("out", [S, H], BF16, kind="ExternalOutput").ap()

    with tile.TileContext(nc) as tc:
        with ExitStack() as ctx:
            E = ctx.enter_context
            persist = E(tc.tile_pool(name="persist", bufs=1))
            psS = E(tc.tile_pool(name="psS", bufs=2, space="PSUM"))
            psQ = E(tc.tile_pool(name="psQ", bufs=1, space="PSUM"))
            psV = E(tc.tile_pool(name="psV", bufs=2, space="PSUM"))
            psO = E(tc.tile_pool(name="psO", bufs=1, space="PSUM"))
            wk = E(tc.tile_pool(name="wk", bufs=4))
            wk2 = E(tc.tile_pool(name="wk2", bufs=2))
            aq = E(tc.tile_pool(name="aq", bufs=8))
            stg = E(tc.tile_pool(name="stg", bufs=3))

            # ---------------- batched loads ----------------
            # DMA device is the startup bottleneck; order/chunk loads so
            # K proj / V proj / first Q projs start as early as possible.
            wk_all = persist.tile([128, NK, KF], BF16, tag="wk_all")
            wkr = wkT.rearrange("(k p) c -> p k c", p=128)
            nc.sync.dma_start(wk_all[:, 0:8, :], wkr[:, 0:8, :])
            xa = persist.tile([128, NK, S], BF16, tag="xa")
            xr = xT.rearrange("(k p) s -> p k s", p=128)
            nc.sync.dma_start(xa[:, 0:4, tbc_(0)], xr[:, 0:4, tbc_(0)])
            nc.sync.dma_start(wk_all[:, 8:16, :], wkr[:, 8:16, :])
            for kc in range(4, NK, 4):
                nc.sync.dma_start(
                    xa[:, kc:kc + 4, tbc_(0)], xr[:, kc:kc + 4, tbc_(0)])
            wv_all = persist.tile([128, NK, KF], BF16, tag="wv_all")
            nc.sync.dma_start(
                wv_all[:], wvT.rearrange("(k p) c -> p k c", p=128))
            rt = persist.tile([128, 128], BF16, tag="rt")
            nc.sync.dma_start(rt[:], rotT[:, :])
            cs = persist.tile([128, S], BF16, tag="cs")
            nc.sync.dma_start(cs[:], cost[:, :])
            sn = persist.tile([128, S], BF16, tag="sn")
            nc.sync.dma_start(sn[:], sint[:, :])
            wq_all = persist.tile([128, NK, QF], BF16, tag="wq_all")
            wqr = wqT.rearrange("(k p) c -> p k c", p=128)
            nc.sync.dma_start(wq_all[:, :, 0:256], wqr[:, :, 0:256])
            tri = persist.tile([128, 128], BF16, tag="tri")
            nc.sync.dma_start(tri[:], trim[:, :])
            nc.sync.dma_start(wq_all[:, :, 256:512], wqr[:, :, 256:512])
            for tb in range(1, NQB):
                nc.sync.dma_start(xa[:, :, tbc_(tb)], xr[:, :, tbc_(tb)])
            wo_sb = []
            for p in range(NPAIR):
                t = persist.tile([128, H], BF16, tag=f"wo{p}", name=f"wo{p}")
                nc.sync.dma_start(t[:], woT[p * 128:(p + 1) * 128, :])
                wo_sb.append(t)

            # ---------------- persistent activation tiles ----------------
            qt_sb = [persist.tile([128, S], BF16, tag=f"qt{p}", name=f"qt{p}") for p in range(NPAIR)]
            ktp = {(sd, v): persist.tile([128, S], BF16, tag=f"ktp{sd}{v}", name=f"ktp{sd}{v}")
                   for sd in (0, 1) for v in (0, 1)}
            va = [persist.tile([128, NT, D + 1], BF16, tag=f"va{v}", name=f"va{v}") for v in (0, 1)]
            att = [persist.tile([128, S], BF16, tag=f"att{p}", name=f"att{p}") for p in range(NPAIR)]

            # zero pads (Pool engine; it is otherwise idle, and these have
            # no input dependencies so they run during the initial loads)
            nc.gpsimd.memset(ktp[(0, 0)][64:128, :], 0.0)
            nc.gpsimd.memset(ktp[(1, 1)][0:64, :], 0.0)
            nc.gpsimd.memset(ktp[(1, 0)][0:64, :], 0.0)
            nc.gpsimd.memset(ktp[(0, 1)][64:128, :], 0.0)
            nc.gpsimd.memset(va[0][:, :, D:D + 1], 1.0)
            nc.gpsimd.memset(va[1][:, :, D:D + 1], 1.0)

            def rope_pre(ps):
                """Drain the projection accumulator to SBUF (releases the
                psQ bank); the rest of RoPE runs in rope_post."""
                raw = wk.tile([128, QBS], BF16, tag="rope_raw")
                nc.vector.tensor_copy(out=raw[:], in_=ps[:])
                return raw

            def rope_post(raw, tb, outs, pool=None, tag="q"):
                rp = (pool or psQ).tile([128, QBS], F32, tag=tag)
                nc.tensor.matmul(rp[:], lhsT=rt[:], rhs=raw[:], start=True, stop=True)
                t1 = wk.tile([128, QBS], BF16, tag="rope_t1")
                nc.vector.tensor_tensor(out=t1[:], in0=rp[:], in1=sn[:, tbc_(tb)], op=OP.mult)
                t2 = wk.tile([128, QBS], BF16, tag="rope_t2")
                nc.vector.tensor_tensor(out=t2[:], in0=raw[:], in1=cs[:, tbc_(tb)], op=OP.mult)
                for rows, out_ap in outs:
                    nc.vector.tensor_tensor(
                        out=out_ap, in0=t1[rows, :], in1=t2[rows, :], op=OP.add)

            def rope(ps, tb, outs):
                rope_post(rope_pre(ps), tb, outs)

            def vproj(t, pool=None, tag="q"):
                vp = (pool or psQ).tile([128, KF], F32, tag=tag)
                for k in range(NK):
                    nc.tensor.matmul(
                        vp[:], lhsT=xa[:, k, t * 128:(t + 1) * 128],
                        rhs=wv_all[:, k, :],
                        start=(k == 0), stop=(k == NK - 1))
                for v in (0, 1):
                    nc.vector.tensor_copy(
                        out=va[v][:, t, 0:D], in_=vp[:, v * D:(v + 1) * D])

            # ---------------- pump machinery ----------------
            # Each pump is a closure emitting ~0.2-0.9us of PE work; pumps
            # are interleaved between attention score groups. All
            # projection PSUM flows through the single psQ bank: within a
            # generator the accumulator's reader (rope raw copy / va copy)
            # is emitted before the next psQ allocation, so the pool
            # rotation never stalls.
            def qproj_pumps(p, tb, pool=None, tag="q"):
                pool = pool or psQ
                st = {}
                def mk(kk):
                    def pump():
                        if kk == 0:
                            st["qp"] = pool.tile([128, QBS], F32, tag=tag,
                                                 name=f"qp{p}_{tb}")
                        for k in (kk, kk + 1):
                            nc.tensor.matmul(
                                st["qp"][:],
                                lhsT=wq_all[:, k, p * 128:(p + 1) * 128],
                                rhs=xa[:, k, tbc_(tb)],
                                start=(k == 0), stop=(k == NK - 1))
                        if kk == NK - 2:
                            st["raw"] = rope_pre(st["qp"])
                    return pump
                pumps = [mk(kk) for kk in range(0, NK, 2)]
                # a score-group slot sits between the drain and the rotate
                # matmul, hiding the DVE copy latency from the PE
                pumps.append(None)
                pumps.append(lambda: rope_post(
                    st["raw"], tb, [(slice(0, 128), qt_sb[p][:, tbc_(tb)])],
                    pool=pool, tag=tag))
                return pumps

            def kproj_pumps(tb, pool=None, tag="q"):
                pool = pool or psQ
                st = {}
                def mk(kk):
                    def pump():
                        if kk == 0:
                            st["kp"] = pool.tile([128, QBS], F32, tag=tag,
                                                 name=f"kp{tb}")
                        for k in range(kk, kk + 4):
                            nc.tensor.matmul(
                                st["kp"][:], lhsT=wk_all[:, k, :],
                                rhs=xa[:, k, tbc_(tb)],
                                start=(k == 0), stop=(k == NK - 1))
                        if kk == NK - 4:
                            st["raw"] = rope_pre(st["kp"])
                    return pump
                pumps = [mk(kk) for kk in range(0, NK, 4)]
                pumps.append(None)
                def fin():
                    rope_post(st["raw"], tb, [
                        (slice(0, 64), ktp[(0, 0)][0:64, tbc_(tb)]),
                        (slice(64, 128), ktp[(1, 1)][64:128, tbc_(tb)]),
                    ], pool=pool, tag=tag)
                    nc.sync.dma_start(ktp[(1, 0)][64:128, tbc_(tb)],
                                      ktp[(0, 0)][0:64, tbc_(tb)])
                    nc.sync.dma_start(ktp[(0, 1)][0:64, tbc_(tb)],
                                      ktp[(1, 1)][64:128, tbc_(tb)])
                pumps.append(fin)
                return pumps

            def oproj_pumps(t, last_split=False, pool=None, pool2=None):
                st = {}
                def mk(n):
                    opool = pool if pool is not None else psO
                    if pool2 is not None and n % 2 == 1:
                        opool = pool2
                    ptag = {id(psO): "op", id(psQ): "q", id(psS): "ps"}[id(opool)]
                    def pump():
                        if n == 0:
                            st["so"] = stg.tile([128, H], BF16, tag="stg",
                                                name=f"so{t}")
                        op_ps = opool.tile([128, 512], F32, tag=ptag)
                        for p2 in range(NPAIR):
                            nc.tensor.matmul(
                                op_ps[:], lhsT=att[p2][:, t * 128:(t + 1) * 128],
                                rhs=wo_sb[p2][:, n * 512:(n + 1) * 512],
                                start=(p2 == 0), stop=(p2 == NPAIR - 1))
                        nc.vector.tensor_copy(
                            out=st["so"][:, n * 512:(n + 1) * 512], in_=op_ps[:])
                        if last_split:
                            nc.sync.dma_start(
                                out[t * 128:(t + 1) * 128,
                                    n * 512:(n + 1) * 512],
                                st["so"][:, n * 512:(n + 1) * 512])
                        elif n == 3:
                            nc.sync.dma_start(
                                out[t * 128:(t + 1) * 128, :], st["so"][:])
                    return pump
                return [mk(n) for n in range(4)]

            def interleave_nonadjacent(big, small):
                """Alternate big (chunky, shared-psum) pumps with small ones
                so consecutive big pumps never contend for their single
                PSUM buffer; pad with no-op spacers once small runs out so
                two bigs never land in the same drain slot."""
                late = 10 ** 9
                res = []
                bi, si = 0, 0
                while bi < len(big) or si < len(small):
                    if si < len(small):
                        res.append(small[si]); si += 1
                    elif bi < len(big) and res and res[-1][0] is not None:
                        res.append((None, late))
                    if bi < len(big):
                        res.append(big[bi]); bi += 1
                return res

            def build_pumps(qb):
                """Returns [(pump, deadline_sg)] for attention block qb.
                Deadlines pin same-block Q pairs before the heads that read
                them; everything else only needs to land by block end."""
                ngrp = 2 * qb + 2
                late = 10 ** 9
                qp = []
                # drains happen at the bottom of each score-group slot, so
                # a deadline of N-1 completes before slot N's score matmul.
                # Generators are joined with spacer slots so the psQ hand-off
                # latency (DVE drain + sem) hides behind a score group.
                # alternate projection accumulators between psQ and psO
                # (psO is free until qb3's o-proj pumps) so consecutive
                # generators' drain chains overlap instead of serializing
                pools = [(psQ, "q"), (psO, "op")] if qb < 3 else [(psQ, "q")]
                psel = [0]
                def nxt():
                    p = pools[psel[0] % len(pools)]
                    psel[0] += 1
                    return p
                gens = []
                if qb == 0:
                    pl, tg = nxt()
                    gens.append([(f, 2 * ngrp - 1)
                                 for f in qproj_pumps(1, 0, pool=pl, tag=tg)])
                pl, tg = nxt()
                gens.append([(f, 4 * ngrp - 1)
                             for f in qproj_pumps(2, qb, pool=pl, tag=tg)])
                if qb + 1 < NQB:
                    pl, tg = nxt()
                    gens.append([(f, late)
                                 for f in kproj_pumps(qb + 1, pool=pl, tag=tg)])
                pl, tg = nxt()
                gens.append([(f, 6 * ngrp - 1)
                             for f in qproj_pumps(3, qb, pool=pl, tag=tg)])
                if qb < 3:
                    vg = []
                    for t in range(4 * qb + 4, 4 * qb + 8):
                        pl, tg = nxt()
                        vg.append((lambda t=t, pl=pl, tg=tg:
                                   vproj(t, pool=pl, tag=tg), late))
                    gens.append(vg)
                if qb + 1 < NQB:
                    pl, tg = nxt()
                    gens.append([(f, late)
                                 for f in qproj_pumps(0, qb + 1, pool=pl, tag=tg)])
                    pl, tg = nxt()
                    gens.append([(f, late)
                                 for f in qproj_pumps(1, qb + 1, pool=pl, tag=tg)])
                for g in gens:
                    if qp:
                        qp.append((None, late))
                    qp += g
                og = []
                if qb == 3:
                    # no more projections in qb3, so psQ is free: alternate
                    # o-proj accumulators between psO and psQ so consecutive
                    # pumps never wait on each other's PSUM drain
                    for t in range(0, 12):
                        og += [(f, late)
                               for f in oproj_pumps(t, pool=psO, pool2=psQ)]
                    return interleave_nonadjacent(qp, og)
                return qp

            # ---------------- phase 0 ----------------
            # K proj block 0 through psS (scores have not started), V tiles
            # 0..3 alternating psS slots while the cos/sin tables are still
            # loading, then K RoPE and the first two Q pairs through psQ.
            kp0 = psS.tile([128, QBS], F32, tag="ps", name="kp0")
            for k in range(NK):
                nc.tensor.matmul(
                    kp0[:], lhsT=wk_all[:, k, :], rhs=xa[:, k, tbc_(0)],
                    start=(k == 0), stop=(k == NK - 1))
            raw0 = rope_pre(kp0)
            for t in range(4):
                vproj(t, pool=psS, tag="ps")
            rope_post(raw0, 0, [
                (slice(0, 64), ktp[(0, 0)][0:64, tbc_(0)]),
                (slice(64, 128), ktp[(1, 1)][64:128, tbc_(0)]),
            ])
            nc.sync.dma_start(ktp[(1, 0)][64:128, tbc_(0)],
                              ktp[(0, 0)][0:64, tbc_(0)])
            nc.sync.dma_start(ktp[(0, 1)][0:64, tbc_(0)],
                              ktp[(1, 1)][64:128, tbc_(0)])
            for pump in qproj_pumps(0, 0):
                if pump is not None:
                    pump()

            # ---------------- attention ----------------
            for qb in range(NQB):
                pumps = build_pumps(qb)
                # suffix-min of deadlines so a due pump forces everything
                # queued in front of it out as well
                due = [0] * (len(pumps) + 1)
                due[len(pumps)] = 10 ** 9
                for i in range(len(pumps) - 1, -1, -1):
                    due[i] = min(pumps[i][1], due[i + 1])
                n_sg = QH * (2 * qb + 2)
                pi = 0
                attq_cur = None
                for hh in range(QH):
                    p = hh // 2
                    half = hh % 2
                    v = hh // 4
                    ksel = ktp[(half, v)]
                    nkv = 4 * qb + 4
                    ngrp = nkv // 2
                    if half == 0:
                        attq_cur = [aq.tile([128, 128], BF16, tag="attq",
                                            name=f"attq{qb}_{p}_{s}")
                                    for s in range(4)]
                    ov = psV.tile([128, 4, D + 1], F32, tag="ov")
                    pts = []
                    for g2 in range(ngrp):
                        sc = psS.tile([128, 2 * QBS], F32, tag="ps")
                        pt = wk.tile([128, 2 * QBS], BF16, tag="pt")
                        j0 = 2 * g2 - 4 * qb
                        for ht in (0, 1):
                            kv = 2 * g2 + ht
                            j = kv - 4 * qb
                            c0 = 128 * j if j > 0 else 0
                            base = ht * QBS
                            nc.tensor.matmul(
                                sc[:, base + c0:base + QBS],
                                lhsT=ksel[:, kv * 128:(kv + 1) * 128],
                                rhs=qt_sb[p][:, qb * QBS + c0:(qb + 1) * QBS],
                                start=True, stop=True)
                        if j0 + 1 < 0:
                            nc.scalar.activation(pt[:], sc[:], AF.Exp)
                        else:
                            for ht in (0, 1):
                                j = 2 * g2 + ht - 4 * qb
                                c0 = 128 * j if j > 0 else 0
                                base = ht * QBS
                                nc.scalar.activation(
                                    pt[:, base + c0:base + QBS],
                                    sc[:, base + c0:base + QBS], AF.Exp)
                        for ht in (0, 1):
                            j = 2 * g2 + ht - 4 * qb
                            if j >= 0:
                                c0 = 128 * j
                                base = ht * QBS
                                nc.vector.tensor_tensor(
                                    out=pt[:, base + c0:base + c0 + 128],
                                    in0=pt[:, base + c0:base + c0 + 128],
                                    in1=tri[:], op=OP.mult)
                        pts.append(pt)
                        if g2 > 0:
                            _pv_group(nc, ov, pts[g2 - 1], va[v], qb, g2 - 1, nkv)
                        # self-correcting pacing: spread the remaining pumps
                        # evenly over the remaining score-group slots, and
                        # force any whose deadline is due
                        sg_idx = hh * ngrp + g2
                        sgs_left = n_sg - sg_idx
                        want = -((pi - len(pumps)) // sgs_left)  # ceil div
                        emitted = 0
                        while pi < len(pumps) and (
                                emitted < want or due[pi] <= sg_idx):
                            if pumps[pi][0] is not None:
                                pumps[pi][0]()
                            pi += 1
                            emitted += 1
                    _pv_group(nc, ov, pts[ngrp - 1], va[v], qb, ngrp - 1, nkv)
                    # normalize: denominator sits at free position 64
                    rec = wk2.tile([128, 4], F32, tag="rec")
                    nc.vector.reciprocal(rec[:, :], ov[:, :, D])
                    for s in range(4):
                        nc.vector.tensor_scalar(
                            out=attq_cur[s][:, half * D:(half + 1) * D],
                            in0=ov[:, s, 0:D],
                            scalar1=rec[:, s:s + 1], scalar2=None, op0=OP.mult)
                    if half == 1:
                        for s in range(4):
                            nc.sync.dma_start_transpose(
                                att[p][:, (qb * 4 + s) * 128:(qb * 4 + s + 1) * 128],
                                attq_cur[s][:])
                while pi < len(pumps):
                    if pumps[pi][0] is not None:
                        pumps[pi][0]()
                    pi += 1
            # trailing o-proj: scores are done, so the psS banks are free
            # and give the accumulators double-buffering; stream the output
            # DMAs per half tile so the final drain is short
            for t in range(12, 16):
                for pump in oproj_pumps(t, last_split=True, pool=psS):
                    pump()
    nc.compile()
    return nc


def _pv_group(nc, ov, pt, vat, qb, g2, nkv):
    """Accumulate PV matmuls for score group g2 (kv tiles 2*g2, 2*g2+1)."""
    for ht in (0, 1):
        kv = 2 * g2 + ht
        j = kv - 4 * qb
        for s in range(4):
            if j > s:
                continue  # this kv tile is fully masked for q subtile s
            nc.tensor.matmul(
                ov[:, s, :],
                lhsT=pt[:, ht * QBS + s * 128:ht * QBS + (s + 1) * 128],
                rhs=vat[:, kv, :],
                start=(kv == 0 and s == 0),
                stop=(kv == nkv - 1 and s == 3))


def _host_tables():
    freq = 1.0 / (10000.0 ** (np.arange(0, D, 2, dtype=np.float64) / D))
    t = np.arange(S, dtype=np.float64)
    fr = t[:, None] * freq[None, :]                       # (S, 32)
    emb = np.concatenate([fr, fr], axis=-1)               # (S, 64)
    cos64 = np.cos(emb).T.astype(np.float32)              # (64, S)
    sin64 = np.sin(emb).T.astype(np.float32)
    cos128 = np.concatenate([cos64, cos64], axis=0).astype(BF)
    sin128 = np.concatenate([sin64, sin64], axis=0).astype(BF)
    R = np.zeros((64, 64), np.float32)
    R[np.arange(32), 32 + np.arange(32)] = -1.0
    R[32 + np.arange(32), np.arange(32)] = 1.0
    R128 = np.zeros((128, 128), np.float32)
    R128[:64, :64] = R
    R128[64:, 64:] = R
    rotT = np.ascontiguousarray(R128.T).astype(BF)
    r = np.arange(128)[:, None]
    c = np.arange(128)[None, :]
    trimask = (r <= c).astype(np.float32).astype(BF)
    return cos128, sin128, rotT, trimask


def _make_in_maps(inputs):
    x = np.asarray(inputs["x"], np.float32)
    Wq = np.asarray(inputs["Wq"], np.float32)
    Wk = np.asarray(inputs["Wk"], np.float32)
    Wv = np.asarray(inputs["Wv"], np.float32)
    Wo = np.asarray(inputs["Wo"], np.float32)
    cos128, sin128, rotT, trimask = _host_tables()
    in_maps = []
    for core in range(8):
        g, b = core // 2, core % 2
        im = {
            "xT": np.ascontiguousarray(x[b].T).astype(BF),
            "wqT": np.ascontiguousarray((Wq[QF * g:QF * (g + 1), :] / 8.0).T).astype(BF),
            "wkT": np.ascontiguousarray(Wk[KF * g:KF * (g + 1), :].T).astype(BF),
            "wvT": np.ascontiguousarray(Wv[KF * g:KF * (g + 1), :].T).astype(BF),
            "woT": np.ascontiguousarray(Wo[:, QF * g:QF * (g + 1)].T).astype(BF),
            "cost": cos128,
            "sint": sin128,
            "rotT": rotT,
            "trim": trimask,
        }
        in_maps.append(im)
    return in_maps


def kernel(x, Wq, Wk, Wv, Wo):
    if "nc" not in _CACHE:
        _CACHE["nc"] = _build_program()
    nc = _CACHE["nc"]

    in_maps = _make_in_maps(
        {"x": x, "Wq": Wq, "Wk": Wk, "Wv": Wv, "Wo": Wo})

    trace = bool(int(os.environ.get("KERNEL_TRACE", "0")))
    res = bass_utils.run_bass_kernel_spmd(
        nc, in_maps, core_ids=list(range(8)), trace=trace)
    _CACHE["last_result"] = res

    out = np.zeros((B, S, H), np.float32)
    for core in range(8):
        g, b = core // 2, core % 2
        out[b] += np.asarray(res.results[core]["out"], np.float32)
    return out
